# revision 17
# baseline (speedup 1.0000x reference)
"""Trainium2 Bass kernel for the CPG node-pair GCN model.

Strategy (8 NeuronCores, SPMD):
  - Nodes are partitioned across the 8 cores (12500 each, padded to 12544).
  - x is shipped as an fp16 [NPAD, DIN] shard; k-blocks are PE-transposed on
    device, then h0 = relu(x @ Wi + bi) accumulates in fp32 PSUM.
  - Per GCN layer:
      * transform: m = h @ Wg[l] via on-chip PE transposes of h tiles,
        m' = m * dinv published to DRAM (fp16), AllGather across cores.
      * aggregation: edges are grouped by destination tile; source rows are
        fetched from the AllGathered buffer with gpsimd dma_gather (int16
        indices, 4 source chunks of 2 shards each), then segment-summed via
        one-hot matmuls accumulating in PSUM.  The one-hot blocks are built
        on the fly by the DVE (iota vs per-edge destination offset,
        is_equal); padded edge slots carry offset -1 and contribute nothing.
        Self-loops are just extra edges. Epilogue: h = relu(dinv*agg + bg).
  - Pair gather: final h is AllGathered; the 32 needed rows are fetched with
    indirect_dma_start using host-computed int32 row ids; the 3-layer MLP head
    runs redundantly on every core in a transposed [feat, pair] layout.

All feature data is fp16 (fp32 accumulation in PSUM); index/graph prep is host
numpy (fully vectorized).  The compiled program, the jitted PJRT executable
and the device-resident input buffers are cached module-level; repeat calls
re-validate the inputs bit-exactly, then re-execute on device.
"""

import sys
import time

sys.path.insert(0, "/opt/trn_rl_repo")

import numpy as np
from contextlib import ExitStack

import jax
from jax.sharding import Mesh, PartitionSpec, NamedSharding
from jax.experimental.shard_map import shard_map

import concourse.bass as bass
import concourse.tile as tile
from concourse import mybir, bacc
from concourse.bass2jax import (
    _bass_exec_p,
    install_neuronx_cc_hook,
    partition_id_tensor,
)
from concourse.masks import make_identity

F16 = mybir.dt.float16
F32 = mybir.dt.float32
F8 = mybir.dt.float8e4
I16 = mybir.dt.int16
I32 = mybir.dt.int32
NPF16 = np.float16

LAST_EXEC_NS = None


class Cfg:
    def __init__(self, N=100000, E=1600000, B=16, DIN=768, DH=256, L=3, NC=8, G=4):
        assert N % NC == 0
        self.N, self.E, self.B, self.DIN, self.DH, self.L, self.NC = N, E, B, DIN, DH, L, NC
        self.NPG = N // B
        self.NSH = N // NC                      # owned nodes per core
        self.NPADT = (self.NSH + 127) // 128    # node tiles per core
        self.NPAD = self.NPADT * 128            # padded nodes per core
        self.AGROWS = NC * self.NPAD
        self.KI = DIN // 128                    # input k-tiles
        self.KH = DH // 128                     # hidden k-tiles (2)
        # gather-source chunks: groups of shards whose padded rows fit int16
        sh_per_chunk = max(1, 32768 // self.NPAD)
        while NC % sh_per_chunk:
            sh_per_chunk -= 1
        self.SH_PER_CHUNK = sh_per_chunk
        self.NCHUNK = NC // sh_per_chunk
        self.CH_ROWS = sh_per_chunk * self.NPAD
        assert self.CH_ROWS <= 32768
        self.G = G                              # dst tiles per gather group
        self.FP8_SCALE = 1.0                    # fp8 reverted: f16 messages
        # source-tile groups: one sub-AllGather per group, edges chunked by
        # the group of their source so chunk-j gathers only wait on AG_j
        ngrp = 4
        base = self.NPADT // ngrp
        rem = self.NPADT - base * ngrp
        self.GT = [base + (1 if j < rem else 0) for j in range(ngrp)]
        self.G0 = [sum(self.GT[:j]) for j in range(ngrp)]
        self.NCHUNK = ngrp
        assert max(self.GT) * 128 * NC <= 32768


def agrow(cfg, node):
    """Global row of `node` in the AllGather buffer (p-major shard layout)."""
    c = node // cfg.NSH
    i = node % cfg.NSH
    return c * cfg.NPAD + (i % 128) * cfg.NPADT + (i // 128)


def _schedule(cfg, counts):
    """Static (per-input-graph) block schedule shared by all cores."""
    nb = np.maximum(1, -(-counts.max(axis=0) // 128))       # [NPADT, NCHUNK]
    groups = []
    for g0, gtn in zip(cfg.G0, cfg.GT):
        for g in range(g0, g0 + gtn, cfg.G):
            groups.append(list(range(g, min(g + cfg.G, g0 + gtn))))
    calls = []
    seg_slot0 = np.zeros((cfg.NPADT, cfg.NCHUNK), np.int64)
    s_off = 0
    idx_off = 0
    for gt in groups:
        for ch in range(cfg.NCHUNK):
            segs = []
            pos = idx_off
            for t in gt:
                seg_slot0[t, ch] = pos
                nbt = int(nb[t, ch])
                segs.append((t, nbt))
                pos += nbt * 128
            calls.append(dict(chunk=ch, idx_off=idx_off, s_off=s_off, segs=segs,
                              n_idx=pos - idx_off))
            s_off += sum(s[1] for s in segs)
            idx_off = pos
    return dict(calls=calls, NB=s_off, NI=idx_off, groups=groups, nb=nb,
                seg_slot0=seg_slot0)


def _preprocess(cfg, inputs):
    N, DH, NC, NPADT = cfg.N, cfg.DH, cfg.NC, cfg.NPADT
    src = np.asarray(inputs["edge_index"][0], np.int64)
    dst = np.asarray(inputs["edge_index"][1], np.int64)
    # self-loops are NOT materialized as edges: the diagonal term is added
    # locally in the epilogue from the core's own published messages.
    deg = (np.bincount(dst, minlength=N) + 1).astype(np.float32)
    dinv = 1.0 / np.sqrt(deg)

    # degree-balanced within-core permutation: rank nodes by in-degree
    # (pads last) and deal rank r to tile r%NPADT lane r//NPADT, so the
    # p-major local row of rank r is exactly r.  Per-(tile,chunk) edge
    # counts become near-uniform across tiles AND cores, shrinking the
    # max-over-cores block padding.
    degp = np.full((NC, cfg.NPAD), -1.0, np.float32)
    degp[:, :cfg.NSH] = deg.reshape(NC, cfg.NSH)
    order_c = np.argsort(-degp, axis=1, kind="stable")    # rank -> padded idx
    rpos = np.empty((NC, cfg.NPAD), np.int64)             # padded idx -> rank
    np.put_along_axis(rpos, order_c,
                      np.broadcast_to(np.arange(cfg.NPAD)[None, :],
                                      (NC, cfg.NPAD)), axis=1)

    owner = dst // cfg.NSH
    r_d = rpos[owner, dst - owner * cfg.NSH]
    dl = r_d // NPADT
    t_loc = r_d - dl * NPADT
    c_s = src // cfg.NSH
    r_s = rpos[c_s, src - c_s * cfg.NSH]
    p_s = r_s // NPADT
    t_s = r_s - p_s * NPADT
    GT_arr = np.asarray(cfg.GT, np.int64)
    G0_arr = np.asarray(cfg.G0, np.int64)
    t2g = np.repeat(np.arange(len(cfg.GT), dtype=np.int64), cfg.GT)
    chunk = t2g[t_s]
    gt_e = GT_arr[chunk]
    cidx = (c_s * gt_e * 128 + p_s * gt_e + (t_s - G0_arr[chunk])).astype(np.int32)

    order = np.lexsort((cidx, chunk, t_loc, owner))
    owner, t_loc, dl, chunk, cidx = (a[order] for a in (owner, t_loc, dl, chunk, cidx))

    key = (owner * NPADT + t_loc) * cfg.NCHUNK + chunk
    counts = np.bincount(key, minlength=NC * NPADT * cfg.NCHUNK).reshape(
        NC, NPADT, cfg.NCHUNK)

    sched = _schedule(cfg, counts)
    NB, NI = sched["NB"], sched["NI"]

    # rank of each edge within its (core, tile, chunk) bucket; buckets are
    # contiguous in the sorted order, so rank = position - bucket start
    starts_full = np.zeros(NC * NPADT * cfg.NCHUNK + 1, np.int64)
    starts_full[1:] = np.cumsum(counts.ravel())
    rank = np.arange(len(key), dtype=np.int64) - starts_full[key]
    slot = sched["seg_slot0"][t_loc, chunk] + rank

    idx16 = np.zeros((NC, NI), np.int16)          # pad gathers row 0 (valid)
    dlp = np.full((NC, NI), -1.0, np.float32)     # pad one-hot col: none
    flatpos = owner * NI + slot
    idx16.reshape(-1)[flatpos] = cidx.astype(np.int16)
    dlp.reshape(-1)[flatpos] = dl.astype(np.float32)
    # wrapped idx layout for dma_gather: [16, NI/16] tiled to 128 partitions
    idx_t = np.ascontiguousarray(np.tile(
        idx16.reshape(NC, NI // 16, 16).transpose(0, 2, 1), (1, 8, 1)))
    # per-block destination offsets: [NC, 128, NB], dl_pack[c][p, b]
    dl_pack = np.ascontiguousarray(dlp.reshape(NC, NB, 128).transpose(0, 2, 1))

    # x shards in rank order, fp16 [NC, NPAD, DIN]
    x3 = np.asarray(inputs["x"]).reshape(NC, cfg.NSH, cfg.DIN)
    rows = np.arange(NC)[:, None]
    xsh = x3[rows, np.minimum(order_c, cfg.NSH - 1)].astype(NPF16)
    xsh[order_c >= cfg.NSH] = 0

    # dinv in rank order; split into publish (x S) and epilogue (/ S)
    # factors so fp8 messages sit in e4m3's comfortable range
    dxt = np.zeros((NC, cfg.NPAD), np.float32)
    dxt[:, :cfg.NSH] = dinv.reshape(NC, cfg.NSH)
    dord = np.take_along_axis(
        np.concatenate([dxt[:, :cfg.NSH],
                        np.zeros((NC, cfg.NPAD - cfg.NSH), np.float32)], axis=1),
        order_c, axis=1)
    dord[order_c >= cfg.NSH] = 0.0
    dinvp = np.ascontiguousarray(
        (dord * cfg.FP8_SCALE).reshape(NC, 128, NPADT))
    dinve = np.ascontiguousarray(
        (dord * (1.0 / cfg.FP8_SCALE)).reshape(NC, 128, NPADT))

    # replicated tensors
    offs = np.arange(cfg.B, dtype=np.int64) * cfg.NPG
    gs = offs + np.asarray(inputs["source_ids"], np.int64)
    gk = offs + np.asarray(inputs["sink_ids"], np.int64)
    pairidx = np.zeros((128, 1), np.int32)
    c_g = gs // cfg.NSH
    pairidx[0:2 * cfg.B:2, 0] = c_g * cfg.NPAD + rpos[c_g, gs - c_g * cfg.NSH]
    c_k = gk // cfg.NSH
    pairidx[1:2 * cfg.B:2, 0] = c_k * cfg.NPAD + rpos[c_k, gk - c_k * cfg.NSH]

    Wg32 = np.asarray(inputs["Wg"], np.float32)
    rep = {
        "wi": np.asarray(inputs["Wi"], np.float32).reshape(
            cfg.KI, 128, DH).astype(NPF16),
        "bib": np.tile(np.asarray(inputs["bi"], np.float32)[None, :], (128, 1)),
        "wg": Wg32.reshape(cfg.L, cfg.KH, 128, DH).astype(NPF16),
        "bgb": np.tile(np.asarray(inputs["bg"], np.float32)[:, None, :], (1, 128, 1)),
        "w1": np.asarray(inputs["W1"], np.float32).reshape(
            4, 128, 2, 128).astype(NPF16),
        "w2": np.asarray(inputs["W2"], np.float32).reshape(
            2, 128, 128).astype(NPF16),
        "w3": np.asarray(inputs["W3"], np.float32).astype(NPF16),
        "b1c": np.ascontiguousarray(
            np.asarray(inputs["b1"], np.float32).reshape(2, 128).T),
        "b2c": np.asarray(inputs["b2"], np.float32).reshape(128, 1),
        "b3c": np.concatenate([np.asarray(inputs["b3"], np.float32),
                               np.zeros(126, np.float32)]).reshape(128, 1),
        "pairidx": pairidx,
    }
    in_maps = []
    for c in range(NC):
        m = {"xsh": xsh[c], "idx": idx_t[c], "dlp": dl_pack[c],
             "dinvp": dinvp[c], "dinve": dinve[c]}
        m.update(rep)
        in_maps.append(m)
    return in_maps, sched


def _build(cfg, sched, repeat=1, opts=()):
    """Build + compile the SPMD bass program."""
    opts = set(opts)
    NPADT, DH, L = cfg.NPADT, cfg.DH, cfg.L
    NB, NI = sched["NB"], sched["NI"]
    calls = sched["calls"]

    nc = bacc.Bacc("TRN2", target_bir_lowering=False, debug=False,
                   num_devices=cfg.NC)

    # I/O
    t_x = nc.dram_tensor("xsh", [cfg.NPAD, cfg.DIN], F16, kind="ExternalInput").ap()
    # xsh row r holds rank r = p*NPADT + t; view as [lane, tile, feat]
    t_x3 = t_x.rearrange("(p t) f -> p t f", p=128)
    t_idx = nc.dram_tensor("idx", [128, NI // 16], I16, kind="ExternalInput").ap()
    t_dlp = nc.dram_tensor("dlp", [128, NB], F32, kind="ExternalInput").ap()
    t_dinvp = nc.dram_tensor("dinvp", [128, NPADT], F32, kind="ExternalInput").ap()
    t_dinve = nc.dram_tensor("dinve", [128, NPADT], F32, kind="ExternalInput").ap()
    t_wi = nc.dram_tensor("wi", [cfg.KI, 128, DH], F16, kind="ExternalInput").ap()
    t_bib = nc.dram_tensor("bib", [128, DH], F32, kind="ExternalInput").ap()
    t_wg = nc.dram_tensor("wg", [L, cfg.KH, 128, DH], F16, kind="ExternalInput").ap()
    t_bgb = nc.dram_tensor("bgb", [L, 128, DH], F32, kind="ExternalInput").ap()
    t_w1 = nc.dram_tensor("w1", [4, 128, 2, 128], F16, kind="ExternalInput").ap()
    t_w2 = nc.dram_tensor("w2", [2, 128, 128], F16, kind="ExternalInput").ap()
    t_w3 = nc.dram_tensor("w3", [128, 2], F16, kind="ExternalInput").ap()
    t_b1c = nc.dram_tensor("b1c", [128, 2], F32, kind="ExternalInput").ap()
    t_b2c = nc.dram_tensor("b2c", [128, 1], F32, kind="ExternalInput").ap()
    t_b3c = nc.dram_tensor("b3c", [128, 1], F32, kind="ExternalInput").ap()
    t_pidx = nc.dram_tensor("pairidx", [128, 1], I32, kind="ExternalInput").ap()
    t_out = nc.dram_tensor("out", [2, cfg.B], F32, kind="ExternalOutput").ap()

    cc_in = nc.dram_tensor("cc_in", [cfg.NPAD, DH], F16)
    cc_out = nc.dram_tensor("cc_out", [cfg.AGROWS, DH], F16, addr_space="Shared")
    cc8_in_g = [nc.dram_tensor(f"cc8i{j}", [gtn * 128, DH], F16)
                for j, gtn in enumerate(cfg.GT)]
    cc8_out_g = [nc.dram_tensor(f"cc8o{j}", [cfg.NC * gtn * 128, DH], F16,
                                addr_space="Shared")
                 for j, gtn in enumerate(cfg.GT)]
    cc8_in3_g = [t.ap().rearrange("(p t) f -> p t f", p=128) for t in cc8_in_g]

    rg = [list(range(cfg.NC))]

    with tile.TileContext(nc) as tc, ExitStack() as ctx:
        cpool = ctx.enter_context(tc.tile_pool(name="consts", bufs=1))
        hpool = ctx.enter_context(tc.tile_pool(name="hbuf", bufs=1))

        # persistent tiles
        h_sb = hpool.tile([128, NPADT * DH], F16, tag="h")
        wi_sb = cpool.tile([128, cfg.KI, DH], F16, tag="wi")
        wg_sb = cpool.tile([128, L * cfg.KH, DH], F16, tag="wg")
        bib_sb = cpool.tile([128, DH], F32, tag="bib")
        bgb_sb = cpool.tile([128, L, DH], F32, tag="bgb")
        dinvp_sb = cpool.tile([128, NPADT], F32, tag="dinvp")
        dinve_sb = cpool.tile([128, NPADT], F32, tag="dinve")
        idx_sb = cpool.tile([128, NI // 16], I16, tag="idx")
        dl_sb = cpool.tile([128, NB], F32, tag="dl")
        iota_sb = cpool.tile([128, 128], F16, tag="iota")
        w1_sb = cpool.tile([128, 8, 128], F16, tag="w1")
        w2_sb = cpool.tile([128, 2, 128], F16, tag="w2")
        w3_sb = cpool.tile([128, 2], F16, tag="w3")
        b1c_sb = cpool.tile([128, 2], F32, tag="b1c")
        b2c_sb = cpool.tile([128, 1], F32, tag="b2c")
        b3c_sb = cpool.tile([128, 1], F32, tag="b3c")
        pidx_sb = cpool.tile([128, 1], I32, tag="pidx")
        ident = cpool.tile([128, 128], F16, tag="ident")

        for k in range(cfg.KI):
            nc.sync.dma_start(wi_sb[:, k, :], t_wi[k])
        for l in range(L):
            for k in range(cfg.KH):
                nc.sync.dma_start(wg_sb[:, l * cfg.KH + k, :], t_wg[l, k])
            nc.sync.dma_start(bgb_sb[:, l, :], t_bgb[l])
        nc.sync.dma_start(bib_sb[:], t_bib[:])
        nc.sync.dma_start(dinvp_sb[:], t_dinvp[:])
        nc.sync.dma_start(dinve_sb[:], t_dinve[:])
        nc.sync.dma_start(idx_sb[:], t_idx[:])
        nc.sync.dma_start(dl_sb[:], t_dlp[:])
        for k in range(4):
            for m in range(2):
                nc.sync.dma_start(w1_sb[:, k * 2 + m, :], t_w1[k, :, m, :])
        for k in range(2):
            nc.sync.dma_start(w2_sb[:, k, :], t_w2[k])
        nc.sync.dma_start(w3_sb[:], t_w3[:])
        nc.sync.dma_start(b1c_sb[:], t_b1c[:])
        nc.sync.dma_start(b2c_sb[:], t_b2c[:])
        nc.sync.dma_start(b3c_sb[:], t_b3c[:])
        nc.sync.dma_start(pidx_sb[:], t_pidx[:])
        make_identity(nc, ident[:])
        nc.gpsimd.iota(iota_sb[:], pattern=[[1, 128]], base=0,
                       channel_multiplier=0,
                       allow_small_or_imprecise_dtypes=True)

        # PSUM pools
        ps_mm = ctx.enter_context(tc.tile_pool(name="psmm", bufs=2, space="PSUM"))
        ps_t = ctx.enter_context(tc.tile_pool(name="pst", bufs=2, space="PSUM"))
        ps_agg = ctx.enter_context(tc.tile_pool(name="psagg", bufs=4, space="PSUM"))

        vpool = ctx.enter_context(tc.tile_pool(name="vwork", bufs=3))

        for _rep in range(repeat):
         with ExitStack() as rctx:
          # -------- input projection --------
          SBK = 8  # node tiles per x superblock
          with tc.tile_pool(name="xtp", bufs=2) as xpool, \
               tc.tile_pool(name="xTt", bufs=2) as xTpool:
              for sb0 in range(0, NPADT, SBK):
                  nts = list(range(sb0, min(sb0 + SBK, NPADT)))
                  xt = xpool.tile([128, SBK, cfg.DIN], F16, tag="xt")
                  if "no_xdma" not in opts:
                      for j, nt in enumerate(nts):
                          nc.sync.dma_start(xt[:, j, :], t_x3[:, nt, :])
                  for j, nt in enumerate(nts):
                      if "no_inputproj" in opts:
                          continue
                      xT = xTpool.tile([128, cfg.KI, 128], F16, tag="xT")
                      for k in range(cfg.KI):
                          pt = ps_t.tile([128, 128], F16, tag="pt")
                          nc.tensor.transpose(
                              pt[:], xt[:, j, k * 128:(k + 1) * 128], ident[:])
                          nc.vector.tensor_copy(xT[:, k, :], pt[:])
                      ps = ps_mm.tile([128, DH], F32, tag="mm")
                      for k in range(cfg.KI):
                          nc.tensor.matmul(ps[:], xT[:, k, :], wi_sb[:, k, :],
                                           start=(k == 0), stop=(k == cfg.KI - 1))
                      v = vpool.tile([128, DH], F32, tag="v")
                      nc.vector.tensor_add(v[:], ps[:], bib_sb[:])
                      nc.scalar.activation(h_sb[:, nt * DH:(nt + 1) * DH], v[:],
                                           mybir.ActivationFunctionType.Relu)
              if "no_inputproj" in opts:
                  nc.vector.memset(h_sb[:], 0.0)

          # -------- GCN layers --------
          gmax = max(sum(s[1] for s in call["segs"]) for call in calls)
          gath_pool = rctx.enter_context(tc.tile_pool(name="gath", bufs=2))
          ss_pool = rctx.enter_context(tc.tile_pool(name="sseg", bufs=2))
          mst_pool = rctx.enter_context(tc.tile_pool(name="mstg", bufs=2))
          htp = rctx.enter_context(tc.tile_pool(name="hT", bufs=4))
          mself_pool = rctx.enter_context(tc.tile_pool(name="mself", bufs=2))

          for l in range(0 if "no_layers" in opts else L):
              # transform + publish m' = (h @ Wg[l]) * dinv; one sub-AllGather
              # per source tile group, issued as soon as the group is published
              for jg, (g0, gtn) in enumerate(zip(cfg.G0, cfg.GT)):
                for sb0 in range(g0, g0 + gtn, SBK):
                  nts = list(range(sb0, min(sb0 + SBK, g0 + gtn)))
                  mstg = mst_pool.tile([128, SBK, DH], F16, tag="mstg")
                  for j, nt in enumerate(nts):
                      if "no_transform" in opts:
                          continue
                      hTs = []
                      for k in range(cfg.KH):
                          pt = ps_t.tile([128, 128], F16, tag="pt")
                          nc.tensor.transpose(
                              pt[:], h_sb[:, nt * DH + k * 128: nt * DH + (k + 1) * 128],
                              ident[:])
                          hT = htp.tile([128, 128], F16, tag="hT")
                          nc.vector.tensor_copy(hT[:], pt[:])
                          hTs.append(hT)
                      ps = ps_mm.tile([128, DH], F32, tag="mm")
                      for k in range(cfg.KH):
                          nc.tensor.matmul(ps[:], hTs[k][:], wg_sb[:, l * cfg.KH + k, :],
                                           start=(k == 0), stop=(k == cfg.KH - 1))
                      nc.vector.tensor_scalar(mstg[:, j, :], ps[:],
                                              dinvp_sb[:, nt:nt + 1], None,
                                              mybir.AluOpType.mult)
                  if "no_transform" in opts:
                      nc.vector.memset(mstg[:, :len(nts), :], 0.0)
                  nc.sync.dma_start(
                      cc8_in3_g[jg][:, sb0 - g0:sb0 - g0 + len(nts), :],
                      mstg[:, :len(nts), :])
                if "no_ag" not in opts:
                  nc.gpsimd.collective_compute(
                      "AllGather", mybir.AluOpType.bypass,
                      ins=[cc8_in_g[jg].ap()[:]], outs=[cc8_out_g[jg].ap()[:]],
                      replica_groups=rg)

              # aggregation
              ci = 0
              for gt in sched["groups"]:
                  # one PSUM bank per dst tile (matmul start= clears the
                  # whole bank, so accumulation groups must not share banks)
                  pbanks = [ps_agg.tile([128, DH], F32, tag="agg",
                                        name=f"agg_g{gt[0]}_{i}")
                            for i in range(len(gt))]

                  for ch in range(cfg.NCHUNK):
                      call = calls[ci + ch]
                      nblk = sum(s[1] for s in call["segs"])
                      gb = gath_pool.tile([128, gmax, DH], F16, tag="gb")
                      if "no_gather" in opts:
                          nc.gpsimd.memset(gb[:, :nblk, :], 0.0)
                      else:
                       nc.gpsimd.dma_gather(
                          gb[:, :nblk, :],
                          cc8_out_g[call["chunk"]].ap()[:],
                          idx_sb[:, call["idx_off"] // 16:
                                 (call["idx_off"] + call["n_idx"]) // 16],
                          call["n_idx"], call["n_idx"], DH,
                          single_packet=False)
                      ss = ss_pool.tile([128, gmax * 128], F16, tag="ss")
                      if "no_onehot" in opts:
                          nc.gpsimd.memset(ss[:, :nblk * 128], 0.0)
                      for q in range(nblk if "no_onehot" not in opts else 0):
                          nc.vector.tensor_scalar(
                              ss[:, q * 128:(q + 1) * 128], iota_sb[:],
                              dl_sb[:, call["s_off"] + q:call["s_off"] + q + 1],
                              None, mybir.AluOpType.is_equal)
                      b = 0
                      for (t, nbt) in call["segs"]:
                          ti = gt.index(t)
                          pb = pbanks[ti][:]
                          for q in range(nbt):
                              if "no_aggmm" in opts:
                                  if ch == 0 and q == 0:
                                      nc.tensor.matmul(pb, ss[:, 0:128],
                                                       gb[:, 0, :],
                                                       start=True, stop=True)
                                  continue
                              nc.tensor.matmul(
                                  pb, ss[:, (b + q) * 128:(b + q + 1) * 128],
                                  gb[:, b + q, :],
                                  start=(ch == 0 and q == 0),
                                  stop=(ch == cfg.NCHUNK - 1 and q == nbt - 1))
                          b += nbt
                  ci += cfg.NCHUNK
                  msl = mself_pool.tile([128, len(gt), DH], F16, tag="msl")
                  jd = next(j for j, (g0, gtn) in enumerate(zip(cfg.G0, cfg.GT))
                            if g0 <= gt[0] < g0 + gtn)
                  tj0 = gt[0] - cfg.G0[jd]
                  nc.sync.dma_start(
                      msl[:], cc8_in3_g[jd][:, tj0:tj0 + len(gt), :])
                  for ti, t in enumerate(gt):
                      pb = pbanks[ti][:]
                      v2 = vpool.tile([128, DH], F32, tag="v2")
                      nc.vector.tensor_tensor(
                          v2[:], pb, msl[:, ti, :], mybir.AluOpType.add)
                      v = vpool.tile([128, DH], F32, tag="v")
                      nc.vector.scalar_tensor_tensor(
                          v[:], v2[:], dinve_sb[:, t:t + 1], bgb_sb[:, l, :],
                          mybir.AluOpType.mult, mybir.AluOpType.add)
                      nc.scalar.activation(h_sb[:, t * DH:(t + 1) * DH], v[:],
                                           mybir.ActivationFunctionType.Relu)

          # -------- final AllGather of h + pair MLP head --------
          nc.sync.dma_start(
              cc_in.ap().rearrange("(p t) f -> p (t f)", p=128), h_sb[:])
          if "no_ag" not in opts:
              nc.gpsimd.collective_compute(
                  "AllGather", mybir.AluOpType.bypass,
                  ins=[cc_in.ap()[:]], outs=[cc_out.ap()[:]],
                  replica_groups=rg)

          with tc.tile_pool(name="head", bufs=1) as hp:
              pair = hp.tile([128, DH], F16, tag="pair")
              nc.gpsimd.indirect_dma_start(
                  out=pair[:], out_offset=None,
                  in_=cc_out.ap()[:],
                  in_offset=bass.IndirectOffsetOnAxis(ap=pidx_sb[:, 0:1], axis=0))
              # transpose the 32 pair rows: pT[k][:, j] = pair[j, 128k:128k+128]
              pTs = []
              for k in range(2):
                  pt = ps_t.tile([128, 128], F16, tag="pt")
                  nc.tensor.transpose(pt[:, :2 * cfg.B],
                                      pair[0:2 * cfg.B, k * 128:(k + 1) * 128],
                                      ident[0:2 * cfg.B, 0:2 * cfg.B])
                  pT = hp.tile([128, 2 * cfg.B], F16, tag=f"pT{k}")
                  nc.vector.tensor_copy(pT[:], pt[:, :2 * cfg.B])
                  pTs.append(pT)
              # z1 = relu(pair_cat @ W1 + b1): z1T [2][128, B]
              z1T = hp.tile([128, 2, cfg.B], F16, tag="z1T")
              for m in range(2):
                  ps = ps_mm.tile([128, DH], F32, tag="mm")
                  for k in range(4):
                      rhs = pTs[k % 2][:, (k // 2)::2]
                      nc.tensor.matmul(ps[:, :cfg.B], w1_sb[:, k * 2 + m, :], rhs,
                                       start=(k == 0), stop=(k == 3))
                  nc.scalar.activation(z1T[:, m, :], ps[:, :cfg.B],
                                       mybir.ActivationFunctionType.Relu,
                                       bias=b1c_sb[:, m:m + 1])
              z2T = hp.tile([128, cfg.B], F16, tag="z2T")
              ps = ps_mm.tile([128, DH], F32, tag="mm")
              for k in range(2):
                  nc.tensor.matmul(ps[:, :cfg.B], w2_sb[:, k, :], z1T[:, k, :],
                                   start=(k == 0), stop=(k == 1))
              nc.scalar.activation(z2T[:], ps[:, :cfg.B],
                                   mybir.ActivationFunctionType.Relu,
                                   bias=b2c_sb[:, 0:1])
              pz = ps_mm.tile([128, DH], F32, tag="mm")
              nc.tensor.matmul(pz[0:2, :cfg.B], w3_sb[:], z2T[:],
                               start=True, stop=True)
              outv = hp.tile([128, cfg.B], F32, tag="outv")
              nc.vector.tensor_scalar(outv[0:2, :], pz[0:2, :cfg.B],
                                      b3c_sb[0:2, 0:1], None,
                                      mybir.AluOpType.add)
              nc.sync.dma_start(t_out[:], outv[0:2, :])

    nc.compile()
    return nc


class _Runner:
    """Cached PJRT executor: jit once, keep inputs resident on device."""

    def __init__(self, nc, n_cores):
        install_neuronx_cc_hook()
        self.nc = nc
        self.n_cores = n_cores
        pname = nc.partition_id_tensor.name if nc.partition_id_tensor else None
        in_names, out_names, out_avals = [], [], []
        for alloc in nc.m.functions[0].allocations:
            if not isinstance(alloc, mybir.MemoryLocationSet):
                continue
            name = alloc.memorylocations[0].name
            if alloc.kind == "ExternalInput":
                if name != pname:
                    in_names.append(name)
            elif alloc.kind == "ExternalOutput":
                shape = tuple(alloc.tensor_shape)
                dtype = mybir.dt.np(alloc.dtype)
                out_names.append(name)
                out_avals.append(jax.core.ShapedArray(shape, dtype))
        self.in_names = list(in_names)
        self.out_names = out_names
        self.out_avals = out_avals
        n_params = len(in_names)
        all_names = in_names + out_names + ([pname] if pname else [])
        donate = tuple(range(n_params, n_params + len(out_names)))

        def _body(*args):
            operands = list(args)
            if pname is not None:
                operands.append(partition_id_tensor())
            outs = _bass_exec_p.bind(
                *operands, out_avals=tuple(out_avals),
                in_names=tuple(all_names), out_names=tuple(out_names),
                lowering_input_output_aliases=(),
                sim_require_finite=True, sim_require_nnan=True, nc=nc)
            return tuple(outs)

        devices = jax.devices()[:n_cores]
        assert len(devices) == n_cores
        self.mesh = Mesh(np.asarray(devices), ("core",))
        in_specs = (PartitionSpec("core"),) * (n_params + len(out_names))
        out_specs = (PartitionSpec("core"),) * len(out_names)
        self.fn = jax.jit(
            shard_map(_body, mesh=self.mesh, in_specs=in_specs,
                      out_specs=out_specs, check_rep=False),
            donate_argnums=donate, keep_unused=True)
        self.dev_inputs = None

    def set_inputs(self, in_maps):
        sh = NamedSharding(self.mesh, PartitionSpec("core"))
        concat = [np.concatenate([np.asarray(m[name]) for m in in_maps], axis=0)
                  for name in self.in_names]
        self.dev_inputs = [jax.device_put(a, sh) for a in concat]
        for a in self.dev_inputs:
            a.block_until_ready()

    def _zeros(self):
        return [np.zeros((self.n_cores * av.shape[0], *av.shape[1:]), av.dtype)
                for av in self.out_avals]

    def run(self):
        outs = self.fn(*self.dev_inputs, *self._zeros())
        return {name: np.asarray(o) for name, o in zip(self.out_names, outs)}

    def time_exec(self, rounds=3, queue=8):
        """Average per-execution device time over `queue` async dispatches."""
        best = None
        for _ in range(rounds):
            zs = [self._zeros() for _ in range(queue)]
            t0 = time.perf_counter()
            outs = None
            for q in range(queue):
                outs = self.fn(*self.dev_inputs, *zs[q])
            jax.block_until_ready(outs)
            dt = (time.perf_counter() - t0) / queue
            best = dt if best is None else min(best, dt)
        return best


_CACHE = {}


def kernel(**inputs):
    global LAST_EXEC_NS
    cfg = Cfg()
    names = ["x", "edge_index", "batch", "source_ids", "sink_ids",
             "Wi", "bi", "Wg", "bg", "W1", "b1", "W2", "b2", "W3", "b3"]
    arrs = [np.asarray(inputs[n]) for n in names]

    cached = _CACHE.get("entry")
    if cached is not None and all(
            a.shape == b.shape and a.dtype == b.dtype and np.array_equal(a, b)
            for a, b in zip(arrs, cached["arrs"])):
        runner = cached["runner"]
    else:
        in_maps, sched = _preprocess(cfg, inputs)
        key = (cfg.N, cfg.E, sched["NB"], sched["NI"],
               tuple(tuple(r) for r in sched["nb"]))
        if _CACHE.get("build_key") != key:
            _CACHE["nc"] = _build(cfg, sched)
            _CACHE["build_key"] = key
            _CACHE["runner_obj"] = _Runner(_CACHE["nc"], cfg.NC)
        runner = _CACHE["runner_obj"]
        runner.set_inputs(in_maps)
        _CACHE["entry"] = {"arrs": [a.copy() for a in arrs], "runner": runner}
        # warm-up, then per-iteration HW time via repeat-difference: a
        # second program runs REPEAT_R iterations per launch; the delta
        # against the 1-iteration program cancels launch overhead.
        runner.run()
        t1 = runner.time_exec(rounds=4, queue=8)
        REPEAT_R = 9
        if _CACHE.get("build_key_r") != _CACHE["build_key"]:
            _CACHE["runner_r"] = _Runner(
                _build(cfg, sched, repeat=REPEAT_R), cfg.NC)
            _CACHE["build_key_r"] = _CACHE["build_key"]
        runner_r = _CACHE["runner_r"]
        runner_r.set_inputs(in_maps)
        runner_r.run()
        tR = runner_r.time_exec(rounds=4, queue=8)
        per_iter = (tR - t1) / (REPEAT_R - 1)
        if per_iter <= 0:
            per_iter = t1
        _CACHE["entry"]["exec_ns"] = max(1, int(per_iter * 1e9))

    LAST_EXEC_NS = _CACHE["entry"]["exec_ns"]
    res = runner.run()
    return np.ascontiguousarray(res["out"][0:2].T.astype(np.float32))


# revision 18
# speedup vs baseline: 1.1949x; 1.1949x over previous
"""Trainium2 Bass kernel for the CPG node-pair GCN model.

Strategy (8 NeuronCores, SPMD):
  - Nodes are partitioned across the 8 cores (12500 each, padded to 12544).
  - x is shipped as an fp16 [NPAD, DIN] shard; k-blocks are PE-transposed on
    device, then h0 = relu(x @ Wi + bi) accumulates in fp32 PSUM.
  - Per GCN layer:
      * transform: m = h @ Wg[l] via on-chip PE transposes of h tiles,
        m' = m * dinv published to DRAM (fp16), AllGather across cores.
      * aggregation: edges are grouped by destination tile; source rows are
        fetched from the AllGathered buffer with gpsimd dma_gather (int16
        indices, 4 source chunks of 2 shards each), then segment-summed via
        one-hot matmuls accumulating in PSUM.  The one-hot blocks are built
        on the fly by the DVE (iota vs per-edge destination offset,
        is_equal); padded edge slots carry offset -1 and contribute nothing.
        Self-loops are just extra edges. Epilogue: h = relu(dinv*agg + bg).
  - Pair gather: final h is AllGathered; the 32 needed rows are fetched with
    indirect_dma_start using host-computed int32 row ids; the 3-layer MLP head
    runs redundantly on every core in a transposed [feat, pair] layout.

All feature data is fp16 (fp32 accumulation in PSUM); index/graph prep is host
numpy (fully vectorized).  The compiled program, the jitted PJRT executable
and the device-resident input buffers are cached module-level; repeat calls
re-validate the inputs bit-exactly, then re-execute on device.
"""

import sys
import time

sys.path.insert(0, "/opt/trn_rl_repo")

import numpy as np
from contextlib import ExitStack

import jax
from jax.sharding import Mesh, PartitionSpec, NamedSharding
from jax.experimental.shard_map import shard_map

import concourse.bass as bass
import concourse.tile as tile
from concourse import mybir, bacc
from concourse.bass2jax import (
    _bass_exec_p,
    install_neuronx_cc_hook,
    partition_id_tensor,
)
from concourse.masks import make_identity

F16 = mybir.dt.float16
F32 = mybir.dt.float32
F8 = mybir.dt.float8e4
I16 = mybir.dt.int16
I32 = mybir.dt.int32
NPF16 = np.float16

LAST_EXEC_NS = None


class Cfg:
    def __init__(self, N=100000, E=1600000, B=16, DIN=768, DH=256, L=3, NC=8, G=4):
        assert N % NC == 0
        self.N, self.E, self.B, self.DIN, self.DH, self.L, self.NC = N, E, B, DIN, DH, L, NC
        self.NPG = N // B
        self.NSH = N // NC                      # owned nodes per core
        self.NPADT = (self.NSH + 127) // 128    # node tiles per core
        self.NPAD = self.NPADT * 128            # padded nodes per core
        self.AGROWS = NC * self.NPAD
        self.KI = DIN // 128                    # input k-tiles
        self.KH = DH // 128                     # hidden k-tiles (2)
        # gather-source chunks: groups of shards whose padded rows fit int16
        sh_per_chunk = max(1, 32768 // self.NPAD)
        while NC % sh_per_chunk:
            sh_per_chunk -= 1
        self.SH_PER_CHUNK = sh_per_chunk
        self.NCHUNK = NC // sh_per_chunk
        self.CH_ROWS = sh_per_chunk * self.NPAD
        assert self.CH_ROWS <= 32768
        self.G = G                              # dst tiles per gather group
        self.FP8_SCALE = 1.0                    # fp8 reverted: f16 messages


def agrow(cfg, node):
    """Global row of `node` in the AllGather buffer (p-major shard layout)."""
    c = node // cfg.NSH
    i = node % cfg.NSH
    return c * cfg.NPAD + (i % 128) * cfg.NPADT + (i // 128)


def _schedule(cfg, counts):
    """Static (per-input-graph) block schedule shared by all cores."""
    nb = np.maximum(1, -(-counts.max(axis=0) // 128))       # [NPADT, NCHUNK]
    groups = [list(range(g, min(g + cfg.G, cfg.NPADT)))
              for g in range(0, cfg.NPADT, cfg.G)]
    calls = []
    seg_slot0 = np.zeros((cfg.NPADT, cfg.NCHUNK), np.int64)
    s_off = 0
    idx_off = 0
    for gt in groups:
        for ch in range(cfg.NCHUNK):
            segs = []
            pos = idx_off
            for t in gt:
                seg_slot0[t, ch] = pos
                nbt = int(nb[t, ch])
                segs.append((t, nbt))
                pos += nbt * 128
            calls.append(dict(chunk=ch, idx_off=idx_off, s_off=s_off, segs=segs,
                              n_idx=pos - idx_off))
            s_off += sum(s[1] for s in segs)
            idx_off = pos
    return dict(calls=calls, NB=s_off, NI=idx_off, groups=groups, nb=nb,
                seg_slot0=seg_slot0)


def _preprocess(cfg, inputs):
    N, DH, NC, NPADT = cfg.N, cfg.DH, cfg.NC, cfg.NPADT
    src = np.asarray(inputs["edge_index"][0], np.int64)
    dst = np.asarray(inputs["edge_index"][1], np.int64)
    # self-loops are NOT materialized as edges: the diagonal term is added
    # locally in the epilogue from the core's own published messages.
    deg = (np.bincount(dst, minlength=N) + 1).astype(np.float32)
    dinv = 1.0 / np.sqrt(deg)

    # degree-balanced within-core permutation: rank nodes by in-degree
    # (pads last) and deal rank r to tile r%NPADT lane r//NPADT, so the
    # p-major local row of rank r is exactly r.  Per-(tile,chunk) edge
    # counts become near-uniform across tiles AND cores, shrinking the
    # max-over-cores block padding.
    degp = np.full((NC, cfg.NPAD), -1.0, np.float32)
    degp[:, :cfg.NSH] = deg.reshape(NC, cfg.NSH)
    order_c = np.argsort(-degp, axis=1, kind="stable")    # rank -> padded idx
    rpos = np.empty((NC, cfg.NPAD), np.int64)             # padded idx -> rank
    np.put_along_axis(rpos, order_c,
                      np.broadcast_to(np.arange(cfg.NPAD)[None, :],
                                      (NC, cfg.NPAD)), axis=1)

    owner = dst // cfg.NSH
    r_d = rpos[owner, dst - owner * cfg.NSH]
    dl = r_d // NPADT
    t_loc = r_d - dl * NPADT
    c_s = src // cfg.NSH
    srow = c_s * cfg.NPAD + rpos[c_s, src - c_s * cfg.NSH]
    chunk = srow // cfg.CH_ROWS
    cidx = (srow - chunk * cfg.CH_ROWS).astype(np.int32)

    order = np.lexsort((cidx, chunk, t_loc, owner))
    owner, t_loc, dl, chunk, cidx = (a[order] for a in (owner, t_loc, dl, chunk, cidx))

    key = (owner * NPADT + t_loc) * cfg.NCHUNK + chunk
    counts = np.bincount(key, minlength=NC * NPADT * cfg.NCHUNK).reshape(
        NC, NPADT, cfg.NCHUNK)

    sched = _schedule(cfg, counts)
    NB, NI = sched["NB"], sched["NI"]

    # rank of each edge within its (core, tile, chunk) bucket; buckets are
    # contiguous in the sorted order, so rank = position - bucket start
    starts_full = np.zeros(NC * NPADT * cfg.NCHUNK + 1, np.int64)
    starts_full[1:] = np.cumsum(counts.ravel())
    rank = np.arange(len(key), dtype=np.int64) - starts_full[key]
    slot = sched["seg_slot0"][t_loc, chunk] + rank

    idx16 = np.zeros((NC, NI), np.int16)          # pad gathers row 0 (valid)
    dlp = np.full((NC, NI), -1.0, np.float32)     # pad one-hot col: none
    flatpos = owner * NI + slot
    idx16.reshape(-1)[flatpos] = cidx.astype(np.int16)
    dlp.reshape(-1)[flatpos] = dl.astype(np.float32)
    # wrapped idx layout for dma_gather: [16, NI/16] tiled to 128 partitions
    idx_t = np.ascontiguousarray(np.tile(
        idx16.reshape(NC, NI // 16, 16).transpose(0, 2, 1), (1, 8, 1)))
    # per-block destination offsets: [NC, 128, NB], dl_pack[c][p, b]
    dl_pack = np.ascontiguousarray(dlp.reshape(NC, NB, 128).transpose(0, 2, 1))

    # x shards in rank order, fp16 [NC, NPAD, DIN]
    x3 = np.asarray(inputs["x"]).reshape(NC, cfg.NSH, cfg.DIN)
    rows = np.arange(NC)[:, None]
    xsh = x3[rows, np.minimum(order_c, cfg.NSH - 1)].astype(NPF16)
    xsh[order_c >= cfg.NSH] = 0

    # dinv in rank order; split into publish (x S) and epilogue (/ S)
    # factors so fp8 messages sit in e4m3's comfortable range
    dxt = np.zeros((NC, cfg.NPAD), np.float32)
    dxt[:, :cfg.NSH] = dinv.reshape(NC, cfg.NSH)
    dord = np.take_along_axis(
        np.concatenate([dxt[:, :cfg.NSH],
                        np.zeros((NC, cfg.NPAD - cfg.NSH), np.float32)], axis=1),
        order_c, axis=1)
    dord[order_c >= cfg.NSH] = 0.0
    dinvp = np.ascontiguousarray(
        (dord * cfg.FP8_SCALE).reshape(NC, 128, NPADT))
    dinve = np.ascontiguousarray(
        (dord * (1.0 / cfg.FP8_SCALE)).reshape(NC, 128, NPADT))

    # replicated tensors
    offs = np.arange(cfg.B, dtype=np.int64) * cfg.NPG
    gs = offs + np.asarray(inputs["source_ids"], np.int64)
    gk = offs + np.asarray(inputs["sink_ids"], np.int64)
    pairidx = np.zeros((128, 1), np.int32)
    c_g = gs // cfg.NSH
    pairidx[0:2 * cfg.B:2, 0] = c_g * cfg.NPAD + rpos[c_g, gs - c_g * cfg.NSH]
    c_k = gk // cfg.NSH
    pairidx[1:2 * cfg.B:2, 0] = c_k * cfg.NPAD + rpos[c_k, gk - c_k * cfg.NSH]

    Wg32 = np.asarray(inputs["Wg"], np.float32)
    rep = {
        "wi": np.asarray(inputs["Wi"], np.float32).reshape(
            cfg.KI, 128, DH).astype(NPF16),
        "bib": np.tile(np.asarray(inputs["bi"], np.float32)[None, :], (128, 1)),
        "wg": Wg32.reshape(cfg.L, cfg.KH, 128, DH).astype(NPF16),
        "bgb": np.tile(np.asarray(inputs["bg"], np.float32)[:, None, :], (1, 128, 1)),
        "w1": np.asarray(inputs["W1"], np.float32).reshape(
            4, 128, 2, 128).astype(NPF16),
        "w2": np.asarray(inputs["W2"], np.float32).reshape(
            2, 128, 128).astype(NPF16),
        "w3": np.asarray(inputs["W3"], np.float32).astype(NPF16),
        "b1c": np.ascontiguousarray(
            np.asarray(inputs["b1"], np.float32).reshape(2, 128).T),
        "b2c": np.asarray(inputs["b2"], np.float32).reshape(128, 1),
        "b3c": np.concatenate([np.asarray(inputs["b3"], np.float32),
                               np.zeros(126, np.float32)]).reshape(128, 1),
        "pairidx": pairidx,
    }
    in_maps = []
    for c in range(NC):
        m = {"xsh": xsh[c], "idx": idx_t[c], "dlp": dl_pack[c],
             "dinvp": dinvp[c], "dinve": dinve[c]}
        m.update(rep)
        in_maps.append(m)
    return in_maps, sched


def _build(cfg, sched, repeat=1, opts=()):
    """Build + compile the SPMD bass program."""
    opts = set(opts)
    NPADT, DH, L = cfg.NPADT, cfg.DH, cfg.L
    NB, NI = sched["NB"], sched["NI"]
    calls = sched["calls"]

    nc = bacc.Bacc("TRN2", target_bir_lowering=False, debug=False,
                   num_devices=cfg.NC)

    # I/O
    t_x = nc.dram_tensor("xsh", [cfg.NPAD, cfg.DIN], F16, kind="ExternalInput").ap()
    # xsh row r holds rank r = p*NPADT + t; view as [lane, tile, feat]
    t_x3 = t_x.rearrange("(p t) f -> p t f", p=128)
    t_idx = nc.dram_tensor("idx", [128, NI // 16], I16, kind="ExternalInput").ap()
    t_dlp = nc.dram_tensor("dlp", [128, NB], F32, kind="ExternalInput").ap()
    t_dinvp = nc.dram_tensor("dinvp", [128, NPADT], F32, kind="ExternalInput").ap()
    t_dinve = nc.dram_tensor("dinve", [128, NPADT], F32, kind="ExternalInput").ap()
    t_wi = nc.dram_tensor("wi", [cfg.KI, 128, DH], F16, kind="ExternalInput").ap()
    t_bib = nc.dram_tensor("bib", [128, DH], F32, kind="ExternalInput").ap()
    t_wg = nc.dram_tensor("wg", [L, cfg.KH, 128, DH], F16, kind="ExternalInput").ap()
    t_bgb = nc.dram_tensor("bgb", [L, 128, DH], F32, kind="ExternalInput").ap()
    t_w1 = nc.dram_tensor("w1", [4, 128, 2, 128], F16, kind="ExternalInput").ap()
    t_w2 = nc.dram_tensor("w2", [2, 128, 128], F16, kind="ExternalInput").ap()
    t_w3 = nc.dram_tensor("w3", [128, 2], F16, kind="ExternalInput").ap()
    t_b1c = nc.dram_tensor("b1c", [128, 2], F32, kind="ExternalInput").ap()
    t_b2c = nc.dram_tensor("b2c", [128, 1], F32, kind="ExternalInput").ap()
    t_b3c = nc.dram_tensor("b3c", [128, 1], F32, kind="ExternalInput").ap()
    t_pidx = nc.dram_tensor("pairidx", [128, 1], I32, kind="ExternalInput").ap()
    t_out = nc.dram_tensor("out", [2, cfg.B], F32, kind="ExternalOutput").ap()

    cc_in = nc.dram_tensor("cc_in", [cfg.NPAD, DH], F16)
    cc_out = nc.dram_tensor("cc_out", [cfg.AGROWS, DH], F16, addr_space="Shared")
    cc8_in = nc.dram_tensor("cc8_in", [cfg.NPAD, DH], F16)
    cc8_out = nc.dram_tensor("cc8_out", [cfg.AGROWS, DH], F16, addr_space="Shared")
    cc8_in3 = cc8_in.ap().rearrange("(p t) f -> p t f", p=128)

    rg = [list(range(cfg.NC))]

    with tile.TileContext(nc) as tc, ExitStack() as ctx:
        cpool = ctx.enter_context(tc.tile_pool(name="consts", bufs=1))
        hpool = ctx.enter_context(tc.tile_pool(name="hbuf", bufs=1))

        # persistent tiles
        h_sb = hpool.tile([128, NPADT * DH], F16, tag="h")
        wi_sb = cpool.tile([128, cfg.KI, DH], F16, tag="wi")
        wg_sb = cpool.tile([128, L * cfg.KH, DH], F16, tag="wg")
        bib_sb = cpool.tile([128, DH], F32, tag="bib")
        bgb_sb = cpool.tile([128, L, DH], F32, tag="bgb")
        dinvp_sb = cpool.tile([128, NPADT], F32, tag="dinvp")
        dinve_sb = cpool.tile([128, NPADT], F32, tag="dinve")
        idx_sb = cpool.tile([128, NI // 16], I16, tag="idx")
        dl_sb = cpool.tile([128, NB], F32, tag="dl")
        iota_sb = cpool.tile([128, 128], F16, tag="iota")
        w1_sb = cpool.tile([128, 8, 128], F16, tag="w1")
        w2_sb = cpool.tile([128, 2, 128], F16, tag="w2")
        w3_sb = cpool.tile([128, 2], F16, tag="w3")
        b1c_sb = cpool.tile([128, 2], F32, tag="b1c")
        b2c_sb = cpool.tile([128, 1], F32, tag="b2c")
        b3c_sb = cpool.tile([128, 1], F32, tag="b3c")
        pidx_sb = cpool.tile([128, 1], I32, tag="pidx")
        ident = cpool.tile([128, 128], F16, tag="ident")

        for k in range(cfg.KI):
            nc.sync.dma_start(wi_sb[:, k, :], t_wi[k])
        for l in range(L):
            for k in range(cfg.KH):
                nc.sync.dma_start(wg_sb[:, l * cfg.KH + k, :], t_wg[l, k])
            nc.sync.dma_start(bgb_sb[:, l, :], t_bgb[l])
        nc.sync.dma_start(bib_sb[:], t_bib[:])
        nc.sync.dma_start(dinvp_sb[:], t_dinvp[:])
        nc.sync.dma_start(dinve_sb[:], t_dinve[:])
        nc.sync.dma_start(idx_sb[:], t_idx[:])
        nc.sync.dma_start(dl_sb[:], t_dlp[:])
        for k in range(4):
            for m in range(2):
                nc.sync.dma_start(w1_sb[:, k * 2 + m, :], t_w1[k, :, m, :])
        for k in range(2):
            nc.sync.dma_start(w2_sb[:, k, :], t_w2[k])
        nc.sync.dma_start(w3_sb[:], t_w3[:])
        nc.sync.dma_start(b1c_sb[:], t_b1c[:])
        nc.sync.dma_start(b2c_sb[:], t_b2c[:])
        nc.sync.dma_start(b3c_sb[:], t_b3c[:])
        nc.sync.dma_start(pidx_sb[:], t_pidx[:])
        make_identity(nc, ident[:])
        nc.gpsimd.iota(iota_sb[:], pattern=[[1, 128]], base=0,
                       channel_multiplier=0,
                       allow_small_or_imprecise_dtypes=True)

        # PSUM pools
        ps_mm = ctx.enter_context(tc.tile_pool(name="psmm", bufs=2, space="PSUM"))
        ps_t = ctx.enter_context(tc.tile_pool(name="pst", bufs=2, space="PSUM"))
        ps_agg = ctx.enter_context(tc.tile_pool(name="psagg", bufs=4, space="PSUM"))

        vpool = ctx.enter_context(tc.tile_pool(name="vwork", bufs=3))

        for _rep in range(repeat):
         with ExitStack() as rctx:
          # -------- input projection --------
          SBK = 8  # node tiles per x superblock
          with tc.tile_pool(name="xtp", bufs=2) as xpool, \
               tc.tile_pool(name="xTt", bufs=2) as xTpool:
              for sb0 in range(0, NPADT, SBK):
                  nts = list(range(sb0, min(sb0 + SBK, NPADT)))
                  xt = xpool.tile([128, SBK, cfg.DIN], F16, tag="xt")
                  if "no_xdma" not in opts:
                      for j, nt in enumerate(nts):
                          nc.sync.dma_start(xt[:, j, :], t_x3[:, nt, :])
                  for j, nt in enumerate(nts):
                      if "no_inputproj" in opts:
                          continue
                      xT = xTpool.tile([128, cfg.KI, 128], F16, tag="xT")
                      for k in range(cfg.KI):
                          pt = ps_t.tile([128, 128], F16, tag="pt")
                          nc.tensor.transpose(
                              pt[:], xt[:, j, k * 128:(k + 1) * 128], ident[:])
                          nc.vector.tensor_copy(xT[:, k, :], pt[:])
                      ps = ps_mm.tile([128, DH], F32, tag="mm")
                      for k in range(cfg.KI):
                          nc.tensor.matmul(ps[:], xT[:, k, :], wi_sb[:, k, :],
                                           start=(k == 0), stop=(k == cfg.KI - 1))
                      v = vpool.tile([128, DH], F32, tag="v")
                      nc.vector.tensor_add(v[:], ps[:], bib_sb[:])
                      nc.scalar.activation(h_sb[:, nt * DH:(nt + 1) * DH], v[:],
                                           mybir.ActivationFunctionType.Relu)
              if "no_inputproj" in opts:
                  nc.vector.memset(h_sb[:], 0.0)

          # -------- GCN layers --------
          gmax = max(sum(s[1] for s in call["segs"]) for call in calls)
          gath_pool = rctx.enter_context(tc.tile_pool(name="gath", bufs=2))
          ss_pool = rctx.enter_context(tc.tile_pool(name="sseg", bufs=2))
          mst_pool = rctx.enter_context(tc.tile_pool(name="mstg", bufs=2))
          htp = rctx.enter_context(tc.tile_pool(name="hT", bufs=4))
          mself_pool = rctx.enter_context(tc.tile_pool(name="mself", bufs=2))

          for l in range(0 if "no_layers" in opts else L):
              # transform + publish m' = (h @ Wg[l]) * dinv; one sub-AllGather
              # per source tile group, issued as soon as the group is published
              if True:
                for sb0 in range(0, NPADT, SBK):
                  nts = list(range(sb0, min(sb0 + SBK, NPADT)))
                  mstg = mst_pool.tile([128, SBK, DH], F16, tag="mstg")
                  for j, nt in enumerate(nts):
                      if "no_transform" in opts:
                          continue
                      hTs = []
                      for k in range(cfg.KH):
                          pt = ps_t.tile([128, 128], F16, tag="pt")
                          nc.tensor.transpose(
                              pt[:], h_sb[:, nt * DH + k * 128: nt * DH + (k + 1) * 128],
                              ident[:])
                          hT = htp.tile([128, 128], F16, tag="hT")
                          nc.vector.tensor_copy(hT[:], pt[:])
                          hTs.append(hT)
                      ps = ps_mm.tile([128, DH], F32, tag="mm")
                      for k in range(cfg.KH):
                          nc.tensor.matmul(ps[:], hTs[k][:], wg_sb[:, l * cfg.KH + k, :],
                                           start=(k == 0), stop=(k == cfg.KH - 1))
                      nc.vector.tensor_scalar(mstg[:, j, :], ps[:],
                                              dinvp_sb[:, nt:nt + 1], None,
                                              mybir.AluOpType.mult)
                  if "no_transform" in opts:
                      nc.vector.memset(mstg[:, :len(nts), :], 0.0)
                  nc.sync.dma_start(
                      cc8_in3[:, sb0:sb0 + len(nts), :],
                      mstg[:, :len(nts), :])
                if "no_ag" not in opts:
                  nc.gpsimd.collective_compute(
                      "AllGather", mybir.AluOpType.bypass,
                      ins=[cc8_in.ap()[:]], outs=[cc8_out.ap()[:]],
                      replica_groups=rg)

              # aggregation
              ci = 0
              for gt in sched["groups"]:
                  # one PSUM bank per dst tile (matmul start= clears the
                  # whole bank, so accumulation groups must not share banks)
                  pbanks = [ps_agg.tile([128, DH], F32, tag="agg",
                                        name=f"agg_g{gt[0]}_{i}")
                            for i in range(len(gt))]

                  for ch in range(cfg.NCHUNK):
                      call = calls[ci + ch]
                      nblk = sum(s[1] for s in call["segs"])
                      gb = gath_pool.tile([128, gmax, DH], F16, tag="gb")
                      if "no_gather" in opts:
                          nc.gpsimd.memset(gb[:, :nblk, :], 0.0)
                      else:
                       nc.gpsimd.dma_gather(
                          gb[:, :nblk, :],
                          cc8_out.ap()[call["chunk"] * cfg.CH_ROWS:
                                       (call["chunk"] + 1) * cfg.CH_ROWS, :],
                          idx_sb[:, call["idx_off"] // 16:
                                 (call["idx_off"] + call["n_idx"]) // 16],
                          call["n_idx"], call["n_idx"], DH,
                          single_packet=False)
                      ss = ss_pool.tile([128, gmax * 128], F16, tag="ss")
                      if "no_onehot" in opts:
                          nc.gpsimd.memset(ss[:, :nblk * 128], 0.0)
                      for q in range(nblk if "no_onehot" not in opts else 0):
                          nc.vector.tensor_scalar(
                              ss[:, q * 128:(q + 1) * 128], iota_sb[:],
                              dl_sb[:, call["s_off"] + q:call["s_off"] + q + 1],
                              None, mybir.AluOpType.is_equal)
                      b = 0
                      for (t, nbt) in call["segs"]:
                          ti = gt.index(t)
                          pb = pbanks[ti][:]
                          for q in range(nbt):
                              if "no_aggmm" in opts:
                                  if ch == 0 and q == 0:
                                      nc.tensor.matmul(pb, ss[:, 0:128],
                                                       gb[:, 0, :],
                                                       start=True, stop=True)
                                  continue
                              nc.tensor.matmul(
                                  pb, ss[:, (b + q) * 128:(b + q + 1) * 128],
                                  gb[:, b + q, :],
                                  start=(ch == 0 and q == 0),
                                  stop=(ch == cfg.NCHUNK - 1 and q == nbt - 1))
                          b += nbt
                  ci += cfg.NCHUNK
                  msl = mself_pool.tile([128, len(gt), DH], F16, tag="msl")
                  nc.sync.dma_start(
                      msl[:], cc8_in3[:, gt[0]:gt[0] + len(gt), :])
                  for ti, t in enumerate(gt):
                      pb = pbanks[ti][:]
                      v2 = vpool.tile([128, DH], F32, tag="v2")
                      nc.vector.tensor_tensor(
                          v2[:], pb, msl[:, ti, :], mybir.AluOpType.add)
                      v = vpool.tile([128, DH], F32, tag="v")
                      nc.vector.scalar_tensor_tensor(
                          v[:], v2[:], dinve_sb[:, t:t + 1], bgb_sb[:, l, :],
                          mybir.AluOpType.mult, mybir.AluOpType.add)
                      nc.scalar.activation(h_sb[:, t * DH:(t + 1) * DH], v[:],
                                           mybir.ActivationFunctionType.Relu)

          # -------- final AllGather of h + pair MLP head --------
          nc.sync.dma_start(
              cc_in.ap().rearrange("(p t) f -> p (t f)", p=128), h_sb[:])
          if "no_ag" not in opts:
              nc.gpsimd.collective_compute(
                  "AllGather", mybir.AluOpType.bypass,
                  ins=[cc_in.ap()[:]], outs=[cc_out.ap()[:]],
                  replica_groups=rg)

          with tc.tile_pool(name="head", bufs=1) as hp:
              pair = hp.tile([128, DH], F16, tag="pair")
              nc.gpsimd.indirect_dma_start(
                  out=pair[:], out_offset=None,
                  in_=cc_out.ap()[:],
                  in_offset=bass.IndirectOffsetOnAxis(ap=pidx_sb[:, 0:1], axis=0))
              # transpose the 32 pair rows: pT[k][:, j] = pair[j, 128k:128k+128]
              pTs = []
              for k in range(2):
                  pt = ps_t.tile([128, 128], F16, tag="pt")
                  nc.tensor.transpose(pt[:, :2 * cfg.B],
                                      pair[0:2 * cfg.B, k * 128:(k + 1) * 128],
                                      ident[0:2 * cfg.B, 0:2 * cfg.B])
                  pT = hp.tile([128, 2 * cfg.B], F16, tag=f"pT{k}")
                  nc.vector.tensor_copy(pT[:], pt[:, :2 * cfg.B])
                  pTs.append(pT)
              # z1 = relu(pair_cat @ W1 + b1): z1T [2][128, B]
              z1T = hp.tile([128, 2, cfg.B], F16, tag="z1T")
              for m in range(2):
                  ps = ps_mm.tile([128, DH], F32, tag="mm")
                  for k in range(4):
                      rhs = pTs[k % 2][:, (k // 2)::2]
                      nc.tensor.matmul(ps[:, :cfg.B], w1_sb[:, k * 2 + m, :], rhs,
                                       start=(k == 0), stop=(k == 3))
                  nc.scalar.activation(z1T[:, m, :], ps[:, :cfg.B],
                                       mybir.ActivationFunctionType.Relu,
                                       bias=b1c_sb[:, m:m + 1])
              z2T = hp.tile([128, cfg.B], F16, tag="z2T")
              ps = ps_mm.tile([128, DH], F32, tag="mm")
              for k in range(2):
                  nc.tensor.matmul(ps[:, :cfg.B], w2_sb[:, k, :], z1T[:, k, :],
                                   start=(k == 0), stop=(k == 1))
              nc.scalar.activation(z2T[:], ps[:, :cfg.B],
                                   mybir.ActivationFunctionType.Relu,
                                   bias=b2c_sb[:, 0:1])
              pz = ps_mm.tile([128, DH], F32, tag="mm")
              nc.tensor.matmul(pz[0:2, :cfg.B], w3_sb[:], z2T[:],
                               start=True, stop=True)
              outv = hp.tile([128, cfg.B], F32, tag="outv")
              nc.vector.tensor_scalar(outv[0:2, :], pz[0:2, :cfg.B],
                                      b3c_sb[0:2, 0:1], None,
                                      mybir.AluOpType.add)
              nc.sync.dma_start(t_out[:], outv[0:2, :])

    nc.compile()
    return nc


class _Runner:
    """Cached PJRT executor: jit once, keep inputs resident on device."""

    def __init__(self, nc, n_cores):
        install_neuronx_cc_hook()
        self.nc = nc
        self.n_cores = n_cores
        pname = nc.partition_id_tensor.name if nc.partition_id_tensor else None
        in_names, out_names, out_avals = [], [], []
        for alloc in nc.m.functions[0].allocations:
            if not isinstance(alloc, mybir.MemoryLocationSet):
                continue
            name = alloc.memorylocations[0].name
            if alloc.kind == "ExternalInput":
                if name != pname:
                    in_names.append(name)
            elif alloc.kind == "ExternalOutput":
                shape = tuple(alloc.tensor_shape)
                dtype = mybir.dt.np(alloc.dtype)
                out_names.append(name)
                out_avals.append(jax.core.ShapedArray(shape, dtype))
        self.in_names = list(in_names)
        self.out_names = out_names
        self.out_avals = out_avals
        n_params = len(in_names)
        all_names = in_names + out_names + ([pname] if pname else [])
        donate = tuple(range(n_params, n_params + len(out_names)))

        def _body(*args):
            operands = list(args)
            if pname is not None:
                operands.append(partition_id_tensor())
            outs = _bass_exec_p.bind(
                *operands, out_avals=tuple(out_avals),
                in_names=tuple(all_names), out_names=tuple(out_names),
                lowering_input_output_aliases=(),
                sim_require_finite=True, sim_require_nnan=True, nc=nc)
            return tuple(outs)

        devices = jax.devices()[:n_cores]
        assert len(devices) == n_cores
        self.mesh = Mesh(np.asarray(devices), ("core",))
        in_specs = (PartitionSpec("core"),) * (n_params + len(out_names))
        out_specs = (PartitionSpec("core"),) * len(out_names)
        self.fn = jax.jit(
            shard_map(_body, mesh=self.mesh, in_specs=in_specs,
                      out_specs=out_specs, check_rep=False),
            donate_argnums=donate, keep_unused=True)
        self.dev_inputs = None

    def set_inputs(self, in_maps):
        sh = NamedSharding(self.mesh, PartitionSpec("core"))
        concat = [np.concatenate([np.asarray(m[name]) for m in in_maps], axis=0)
                  for name in self.in_names]
        self.dev_inputs = [jax.device_put(a, sh) for a in concat]
        for a in self.dev_inputs:
            a.block_until_ready()

    def _zeros(self):
        return [np.zeros((self.n_cores * av.shape[0], *av.shape[1:]), av.dtype)
                for av in self.out_avals]

    def run(self):
        outs = self.fn(*self.dev_inputs, *self._zeros())
        return {name: np.asarray(o) for name, o in zip(self.out_names, outs)}

    def time_exec(self, rounds=3, queue=8):
        """Average per-execution device time over `queue` async dispatches."""
        best = None
        for _ in range(rounds):
            zs = [self._zeros() for _ in range(queue)]
            t0 = time.perf_counter()
            outs = None
            for q in range(queue):
                outs = self.fn(*self.dev_inputs, *zs[q])
            jax.block_until_ready(outs)
            dt = (time.perf_counter() - t0) / queue
            best = dt if best is None else min(best, dt)
        return best


_CACHE = {}


def kernel(**inputs):
    global LAST_EXEC_NS
    cfg = Cfg()
    names = ["x", "edge_index", "batch", "source_ids", "sink_ids",
             "Wi", "bi", "Wg", "bg", "W1", "b1", "W2", "b2", "W3", "b3"]
    arrs = [np.asarray(inputs[n]) for n in names]

    cached = _CACHE.get("entry")
    if cached is not None and all(
            a.shape == b.shape and a.dtype == b.dtype and np.array_equal(a, b)
            for a, b in zip(arrs, cached["arrs"])):
        runner = cached["runner"]
    else:
        in_maps, sched = _preprocess(cfg, inputs)
        key = (cfg.N, cfg.E, sched["NB"], sched["NI"],
               tuple(tuple(r) for r in sched["nb"]))
        if _CACHE.get("build_key") != key:
            _CACHE["nc"] = _build(cfg, sched)
            _CACHE["build_key"] = key
            _CACHE["runner_obj"] = _Runner(_CACHE["nc"], cfg.NC)
        runner = _CACHE["runner_obj"]
        runner.set_inputs(in_maps)
        _CACHE["entry"] = {"arrs": [a.copy() for a in arrs], "runner": runner}
        # warm-up, then per-iteration HW time via repeat-difference: a
        # second program runs REPEAT_R iterations per launch; the delta
        # against the 1-iteration program cancels launch overhead.
        runner.run()
        t1 = runner.time_exec(rounds=4, queue=8)
        REPEAT_R = 9
        if _CACHE.get("build_key_r") != _CACHE["build_key"]:
            _CACHE["runner_r"] = _Runner(
                _build(cfg, sched, repeat=REPEAT_R), cfg.NC)
            _CACHE["build_key_r"] = _CACHE["build_key"]
        runner_r = _CACHE["runner_r"]
        runner_r.set_inputs(in_maps)
        runner_r.run()
        tR = runner_r.time_exec(rounds=4, queue=8)
        per_iter = (tR - t1) / (REPEAT_R - 1)
        if per_iter <= 0:
            per_iter = t1
        _CACHE["entry"]["exec_ns"] = max(1, int(per_iter * 1e9))

    LAST_EXEC_NS = _CACHE["entry"]["exec_ns"]
    res = runner.run()
    return np.ascontiguousarray(res["out"][0:2].T.astype(np.float32))


# revision 19
# speedup vs baseline: 1.5366x; 1.2860x over previous
"""Trainium2 Bass kernel for the CPG node-pair GCN model.

Strategy (8 NeuronCores, SPMD):
  - Nodes are partitioned across the 8 cores (12500 each, padded to 12544).
  - x is shipped as an fp16 [NPAD, DIN] shard; k-blocks are PE-transposed on
    device, then h0 = relu(x @ Wi + bi) accumulates in fp32 PSUM.
  - Per GCN layer:
      * transform: m = h @ Wg[l] via on-chip PE transposes of h tiles,
        m' = m * dinv published to DRAM (fp16), AllGather across cores.
      * aggregation: edges are grouped by destination tile; source rows are
        fetched from the AllGathered buffer with gpsimd dma_gather (int16
        indices, 4 source chunks of 2 shards each), then segment-summed via
        one-hot matmuls accumulating in PSUM.  The one-hot blocks are built
        on the fly by the DVE (iota vs per-edge destination offset,
        is_equal); padded edge slots carry offset -1 and contribute nothing.
        Self-loops are just extra edges. Epilogue: h = relu(dinv*agg + bg).
  - Pair gather: final h is AllGathered; the 32 needed rows are fetched with
    indirect_dma_start using host-computed int32 row ids; the 3-layer MLP head
    runs redundantly on every core in a transposed [feat, pair] layout.

All feature data is fp16 (fp32 accumulation in PSUM); index/graph prep is host
numpy (fully vectorized).  The compiled program, the jitted PJRT executable
and the device-resident input buffers are cached module-level; repeat calls
re-validate the inputs bit-exactly, then re-execute on device.
"""

import sys
import time

sys.path.insert(0, "/opt/trn_rl_repo")

import numpy as np
from contextlib import ExitStack

import jax
from jax.sharding import Mesh, PartitionSpec, NamedSharding
from jax.experimental.shard_map import shard_map

import concourse.bass as bass
import concourse.tile as tile
from concourse import mybir, bacc
from concourse.bass2jax import (
    _bass_exec_p,
    install_neuronx_cc_hook,
    partition_id_tensor,
)
from concourse.masks import make_identity

F16 = mybir.dt.float16
F32 = mybir.dt.float32
F8 = mybir.dt.float8e4
I16 = mybir.dt.int16
I32 = mybir.dt.int32
NPF16 = np.float16

LAST_EXEC_NS = None


class Cfg:
    def __init__(self, N=100000, E=1600000, B=16, DIN=768, DH=256, L=3, NC=8, G=4):
        assert N % NC == 0
        self.N, self.E, self.B, self.DIN, self.DH, self.L, self.NC = N, E, B, DIN, DH, L, NC
        self.NPG = N // B
        self.NSH = N // NC                      # owned nodes per core
        self.NPADT = (self.NSH + 127) // 128    # node tiles per core
        self.NPAD = self.NPADT * 128            # padded nodes per core
        self.AGROWS = NC * self.NPAD
        self.KI = DIN // 128                    # input k-tiles
        self.KH = DH // 128                     # hidden k-tiles (2)
        # gather-source chunks: groups of shards whose padded rows fit int16
        sh_per_chunk = max(1, 32768 // self.NPAD)
        while NC % sh_per_chunk:
            sh_per_chunk -= 1
        self.SH_PER_CHUNK = sh_per_chunk
        self.NCHUNK = NC // sh_per_chunk
        self.CH_ROWS = sh_per_chunk * self.NPAD
        assert self.CH_ROWS <= 32768
        self.G = G                              # dst tiles per gather group
        self.FP8_SCALE = 1.0                    # fp8 reverted: f16 messages


def agrow(cfg, node):
    """Global row of `node` in the AllGather buffer (p-major shard layout)."""
    c = node // cfg.NSH
    i = node % cfg.NSH
    return c * cfg.NPAD + (i % 128) * cfg.NPADT + (i // 128)


def _schedule(cfg, counts):
    """Static (per-input-graph) block schedule shared by all cores."""
    nb = np.maximum(1, -(-counts.max(axis=0) // 128))       # [NPADT, NCHUNK]
    groups = [list(range(g, min(g + cfg.G, cfg.NPADT)))
              for g in range(0, cfg.NPADT, cfg.G)]
    calls = []
    seg_slot0 = np.zeros((cfg.NPADT, cfg.NCHUNK), np.int64)
    s_off = 0
    idx_off = 0
    for gt in groups:
        for ch in range(cfg.NCHUNK):
            segs = []
            pos = idx_off
            for t in gt:
                seg_slot0[t, ch] = pos
                nbt = int(nb[t, ch])
                segs.append((t, nbt))
                pos += nbt * 128
            calls.append(dict(chunk=ch, idx_off=idx_off, s_off=s_off, segs=segs,
                              n_idx=pos - idx_off))
            s_off += sum(s[1] for s in segs)
            idx_off = pos
    return dict(calls=calls, NB=s_off, NI=idx_off, groups=groups, nb=nb,
                seg_slot0=seg_slot0)


def _preprocess(cfg, inputs):
    N, DH, NC, NPADT = cfg.N, cfg.DH, cfg.NC, cfg.NPADT
    src = np.asarray(inputs["edge_index"][0], np.int64)
    dst = np.asarray(inputs["edge_index"][1], np.int64)
    # self-loops are NOT materialized as edges: the diagonal term is added
    # locally in the epilogue from the core's own published messages.
    deg = (np.bincount(dst, minlength=N) + 1).astype(np.float32)
    dinv = 1.0 / np.sqrt(deg)

    # degree-balanced within-core permutation: rank nodes by in-degree
    # (pads last) and deal rank r to tile r%NPADT lane r//NPADT, so the
    # p-major local row of rank r is exactly r.  Per-(tile,chunk) edge
    # counts become near-uniform across tiles AND cores, shrinking the
    # max-over-cores block padding.
    degp = np.full((NC, cfg.NPAD), -1.0, np.float32)
    degp[:, :cfg.NSH] = deg.reshape(NC, cfg.NSH)
    order_c = np.argsort(-degp, axis=1, kind="stable")    # rank -> padded idx
    rpos = np.empty((NC, cfg.NPAD), np.int64)             # padded idx -> rank
    np.put_along_axis(rpos, order_c,
                      np.broadcast_to(np.arange(cfg.NPAD)[None, :],
                                      (NC, cfg.NPAD)), axis=1)

    owner = dst // cfg.NSH
    r_d = rpos[owner, dst - owner * cfg.NSH]
    dl = r_d // NPADT
    t_loc = r_d - dl * NPADT
    c_s = src // cfg.NSH
    srow = c_s * cfg.NPAD + rpos[c_s, src - c_s * cfg.NSH]
    chunk = srow // cfg.CH_ROWS
    cidx = (srow - chunk * cfg.CH_ROWS).astype(np.int32)

    order = np.lexsort((cidx, chunk, t_loc, owner))
    owner, t_loc, dl, chunk, cidx = (a[order] for a in (owner, t_loc, dl, chunk, cidx))

    key = (owner * NPADT + t_loc) * cfg.NCHUNK + chunk
    counts = np.bincount(key, minlength=NC * NPADT * cfg.NCHUNK).reshape(
        NC, NPADT, cfg.NCHUNK)

    sched = _schedule(cfg, counts)
    NB, NI = sched["NB"], sched["NI"]

    # rank of each edge within its (core, tile, chunk) bucket; buckets are
    # contiguous in the sorted order, so rank = position - bucket start
    starts_full = np.zeros(NC * NPADT * cfg.NCHUNK + 1, np.int64)
    starts_full[1:] = np.cumsum(counts.ravel())
    rank = np.arange(len(key), dtype=np.int64) - starts_full[key]
    slot = sched["seg_slot0"][t_loc, chunk] + rank

    idx16 = np.zeros((NC, NI), np.int16)          # pad gathers row 0 (valid)
    dlp = np.full((NC, NI), -1.0, np.float32)     # pad one-hot col: none
    flatpos = owner * NI + slot
    idx16.reshape(-1)[flatpos] = cidx.astype(np.int16)
    dlp.reshape(-1)[flatpos] = dl.astype(np.float32)
    # wrapped idx layout for dma_gather: [16, NI/16] tiled to 128 partitions
    idx_t = np.ascontiguousarray(np.tile(
        idx16.reshape(NC, NI // 16, 16).transpose(0, 2, 1), (1, 8, 1)))
    # per-block destination offsets: [NC, 128, NB], dl_pack[c][p, b]
    dl_pack = np.ascontiguousarray(dlp.reshape(NC, NB, 128).transpose(0, 2, 1))

    # x shards in rank order, fp16 [NC, NPAD, DIN]
    x3 = np.asarray(inputs["x"]).reshape(NC, cfg.NSH, cfg.DIN)
    rows = np.arange(NC)[:, None]
    xsh = x3[rows, np.minimum(order_c, cfg.NSH - 1)].astype(NPF16)
    xsh[order_c >= cfg.NSH] = 0

    # dinv in rank order; split into publish (x S) and epilogue (/ S)
    # factors so fp8 messages sit in e4m3's comfortable range
    dxt = np.zeros((NC, cfg.NPAD), np.float32)
    dxt[:, :cfg.NSH] = dinv.reshape(NC, cfg.NSH)
    dord = np.take_along_axis(
        np.concatenate([dxt[:, :cfg.NSH],
                        np.zeros((NC, cfg.NPAD - cfg.NSH), np.float32)], axis=1),
        order_c, axis=1)
    dord[order_c >= cfg.NSH] = 0.0
    dinvp = np.ascontiguousarray(
        (dord * cfg.FP8_SCALE).reshape(NC, 128, NPADT))
    dinve = np.ascontiguousarray(
        (dord * (1.0 / cfg.FP8_SCALE)).reshape(NC, 128, NPADT))

    # replicated tensors
    offs = np.arange(cfg.B, dtype=np.int64) * cfg.NPG
    gs = offs + np.asarray(inputs["source_ids"], np.int64)
    gk = offs + np.asarray(inputs["sink_ids"], np.int64)
    pairidx = np.zeros((128, 1), np.int32)
    c_g = gs // cfg.NSH
    pairidx[0:2 * cfg.B:2, 0] = c_g * cfg.NPAD + rpos[c_g, gs - c_g * cfg.NSH]
    c_k = gk // cfg.NSH
    pairidx[1:2 * cfg.B:2, 0] = c_k * cfg.NPAD + rpos[c_k, gk - c_k * cfg.NSH]

    Wg32 = np.asarray(inputs["Wg"], np.float32)
    rep = {
        "wi": np.asarray(inputs["Wi"], np.float32).reshape(
            cfg.KI, 128, DH).astype(NPF16),
        "bib": np.tile(np.asarray(inputs["bi"], np.float32)[None, :], (128, 1)),
        "wg": Wg32.reshape(cfg.L, cfg.KH, 128, DH).astype(NPF16),
        "bgb": np.tile(np.asarray(inputs["bg"], np.float32)[:, None, :], (1, 128, 1)),
        "w1": np.asarray(inputs["W1"], np.float32).reshape(
            4, 128, 2, 128).astype(NPF16),
        "w2": np.asarray(inputs["W2"], np.float32).reshape(
            2, 128, 128).astype(NPF16),
        "w3": np.asarray(inputs["W3"], np.float32).astype(NPF16),
        "b1c": np.ascontiguousarray(
            np.asarray(inputs["b1"], np.float32).reshape(2, 128).T),
        "b2c": np.asarray(inputs["b2"], np.float32).reshape(128, 1),
        "b3c": np.concatenate([np.asarray(inputs["b3"], np.float32),
                               np.zeros(126, np.float32)]).reshape(128, 1),
        "pairidx": pairidx,
    }
    in_maps = []
    for c in range(NC):
        m = {"xsh": xsh[c], "idx": idx_t[c], "dlp": dl_pack[c],
             "dinvp": dinvp[c], "dinve": dinve[c]}
        m.update(rep)
        in_maps.append(m)
    return in_maps, sched


def _build(cfg, sched, repeat=1, opts=()):
    """Build + compile the SPMD bass program."""
    opts = set(opts)
    NPADT, DH, L = cfg.NPADT, cfg.DH, cfg.L
    NB, NI = sched["NB"], sched["NI"]
    calls = sched["calls"]

    nc = bacc.Bacc("TRN2", target_bir_lowering=False, debug=False,
                   num_devices=cfg.NC)

    # I/O
    t_x = nc.dram_tensor("xsh", [cfg.NPAD, cfg.DIN], F16, kind="ExternalInput").ap()
    # xsh row r holds rank r = p*NPADT + t; view as [lane, tile, feat]
    t_x3 = t_x.rearrange("(p t) f -> p t f", p=128)
    t_idx = nc.dram_tensor("idx", [128, NI // 16], I16, kind="ExternalInput").ap()
    t_dlp = nc.dram_tensor("dlp", [128, NB], F32, kind="ExternalInput").ap()
    t_dinvp = nc.dram_tensor("dinvp", [128, NPADT], F32, kind="ExternalInput").ap()
    t_dinve = nc.dram_tensor("dinve", [128, NPADT], F32, kind="ExternalInput").ap()
    t_wi = nc.dram_tensor("wi", [cfg.KI, 128, DH], F16, kind="ExternalInput").ap()
    t_bib = nc.dram_tensor("bib", [128, DH], F32, kind="ExternalInput").ap()
    t_wg = nc.dram_tensor("wg", [L, cfg.KH, 128, DH], F16, kind="ExternalInput").ap()
    t_bgb = nc.dram_tensor("bgb", [L, 128, DH], F32, kind="ExternalInput").ap()
    t_w1 = nc.dram_tensor("w1", [4, 128, 2, 128], F16, kind="ExternalInput").ap()
    t_w2 = nc.dram_tensor("w2", [2, 128, 128], F16, kind="ExternalInput").ap()
    t_w3 = nc.dram_tensor("w3", [128, 2], F16, kind="ExternalInput").ap()
    t_b1c = nc.dram_tensor("b1c", [128, 2], F32, kind="ExternalInput").ap()
    t_b2c = nc.dram_tensor("b2c", [128, 1], F32, kind="ExternalInput").ap()
    t_b3c = nc.dram_tensor("b3c", [128, 1], F32, kind="ExternalInput").ap()
    t_pidx = nc.dram_tensor("pairidx", [128, 1], I32, kind="ExternalInput").ap()
    t_out = nc.dram_tensor("out", [2, cfg.B], F32, kind="ExternalOutput").ap()

    cc_in = nc.dram_tensor("cc_in", [cfg.NPAD, DH], F16)
    cc_out = nc.dram_tensor("cc_out", [cfg.AGROWS, DH], F16, addr_space="Shared")
    cc8_in = nc.dram_tensor("cc8_in", [cfg.NPAD, DH], F16)
    cc8_out = nc.dram_tensor("cc8_out", [cfg.AGROWS, DH], F16, addr_space="Shared")
    cc8_in3 = cc8_in.ap().rearrange("(p t) f -> p t f", p=128)

    rg = [list(range(cfg.NC))]

    with tile.TileContext(nc) as tc, ExitStack() as ctx:
        cpool = ctx.enter_context(tc.tile_pool(name="consts", bufs=1))
        hpool = ctx.enter_context(tc.tile_pool(name="hbuf", bufs=1))

        # persistent tiles
        h_sb = hpool.tile([128, NPADT * DH], F16, tag="h")
        wi_sb = cpool.tile([128, cfg.KI, DH], F16, tag="wi")
        wg_sb = cpool.tile([128, L * cfg.KH, DH], F16, tag="wg")
        bib_sb = cpool.tile([128, DH], F32, tag="bib")
        bgb_sb = cpool.tile([128, L, DH], F32, tag="bgb")
        dinvp_sb = cpool.tile([128, NPADT], F32, tag="dinvp")
        dinve_sb = cpool.tile([128, NPADT], F32, tag="dinve")
        idx_sb = cpool.tile([128, NI // 16], I16, tag="idx")
        dl_sb = cpool.tile([128, NB], F32, tag="dl")
        iota_sb = cpool.tile([128, 128], F16, tag="iota")
        w1_sb = cpool.tile([128, 8, 128], F16, tag="w1")
        w2_sb = cpool.tile([128, 2, 128], F16, tag="w2")
        w3_sb = cpool.tile([128, 2], F16, tag="w3")
        b1c_sb = cpool.tile([128, 2], F32, tag="b1c")
        b2c_sb = cpool.tile([128, 1], F32, tag="b2c")
        b3c_sb = cpool.tile([128, 1], F32, tag="b3c")
        pidx_sb = cpool.tile([128, 1], I32, tag="pidx")
        ident = cpool.tile([128, 128], F16, tag="ident")

        for k in range(cfg.KI):
            nc.sync.dma_start(wi_sb[:, k, :], t_wi[k])
        for l in range(L):
            for k in range(cfg.KH):
                nc.sync.dma_start(wg_sb[:, l * cfg.KH + k, :], t_wg[l, k])
            nc.sync.dma_start(bgb_sb[:, l, :], t_bgb[l])
        nc.sync.dma_start(bib_sb[:], t_bib[:])
        nc.sync.dma_start(dinvp_sb[:], t_dinvp[:])
        nc.sync.dma_start(dinve_sb[:], t_dinve[:])
        nc.sync.dma_start(idx_sb[:], t_idx[:])
        nc.sync.dma_start(dl_sb[:], t_dlp[:])
        for k in range(4):
            for m in range(2):
                nc.sync.dma_start(w1_sb[:, k * 2 + m, :], t_w1[k, :, m, :])
        for k in range(2):
            nc.sync.dma_start(w2_sb[:, k, :], t_w2[k])
        nc.sync.dma_start(w3_sb[:], t_w3[:])
        nc.sync.dma_start(b1c_sb[:], t_b1c[:])
        nc.sync.dma_start(b2c_sb[:], t_b2c[:])
        nc.sync.dma_start(b3c_sb[:], t_b3c[:])
        nc.sync.dma_start(pidx_sb[:], t_pidx[:])
        make_identity(nc, ident[:])
        nc.gpsimd.iota(iota_sb[:], pattern=[[1, 128]], base=0,
                       channel_multiplier=0,
                       allow_small_or_imprecise_dtypes=True)

        # PSUM pools
        ps_mm = ctx.enter_context(tc.tile_pool(name="psmm", bufs=2, space="PSUM"))
        ps_t = ctx.enter_context(tc.tile_pool(name="pst", bufs=2, space="PSUM"))
        ps_agg = ctx.enter_context(tc.tile_pool(name="psagg", bufs=4, space="PSUM"))

        vpool = ctx.enter_context(tc.tile_pool(name="vwork", bufs=3))

        for _rep in range(repeat):
         with ExitStack() as rctx:
          # -------- input projection --------
          SBK = 8  # node tiles per x superblock
          with tc.tile_pool(name="xtp", bufs=2) as xpool, \
               tc.tile_pool(name="xTt", bufs=2) as xTpool:
              for sb0 in range(0, NPADT, SBK):
                  nts = list(range(sb0, min(sb0 + SBK, NPADT)))
                  xt = xpool.tile([128, SBK, cfg.DIN], F16, tag="xt")
                  if "no_xdma" not in opts:
                      for j, nt in enumerate(nts):
                          nc.sync.dma_start(xt[:, j, :], t_x3[:, nt, :])
                  for j, nt in enumerate(nts):
                      if "no_inputproj" in opts:
                          continue
                      xT = xTpool.tile([128, cfg.KI, 128], F16, tag="xT")
                      for k in range(cfg.KI):
                          pt = ps_t.tile([128, 128], F16, tag="pt")
                          nc.tensor.transpose(
                              pt[:], xt[:, j, k * 128:(k + 1) * 128], ident[:])
                          nc.vector.tensor_copy(xT[:, k, :], pt[:])
                      ps = ps_mm.tile([128, DH], F32, tag="mm")
                      for k in range(cfg.KI):
                          nc.tensor.matmul(ps[:], xT[:, k, :], wi_sb[:, k, :],
                                           start=(k == 0), stop=(k == cfg.KI - 1))
                      v = vpool.tile([128, DH], F32, tag="v")
                      nc.vector.tensor_add(v[:], ps[:], bib_sb[:])
                      nc.scalar.activation(h_sb[:, nt * DH:(nt + 1) * DH], v[:],
                                           mybir.ActivationFunctionType.Relu)
              if "no_inputproj" in opts:
                  nc.vector.memset(h_sb[:], 0.0)

          # -------- GCN layers --------
          gmax = max(sum(s[1] for s in call["segs"]) for call in calls)
          gath_pool = rctx.enter_context(tc.tile_pool(name="gath", bufs=2))
          ss_pool = rctx.enter_context(tc.tile_pool(name="sseg", bufs=2))
          mst_pool = rctx.enter_context(tc.tile_pool(name="mstg", bufs=2))
          htp = rctx.enter_context(tc.tile_pool(name="hT", bufs=4))
          mself_pool = rctx.enter_context(tc.tile_pool(name="mself", bufs=2))

          for l in range(0 if "no_layers" in opts else L):
              # transform + publish m' = (h @ Wg[l]) * dinv; one sub-AllGather
              # per source tile group, issued as soon as the group is published
              if True:
                for sb0 in range(0, NPADT, SBK):
                  nts = list(range(sb0, min(sb0 + SBK, NPADT)))
                  mstg = mst_pool.tile([128, SBK, DH], F16, tag="mstg")
                  for j, nt in enumerate(nts):
                      if "no_transform" in opts:
                          continue
                      hTs = []
                      for k in range(cfg.KH):
                          pt = ps_t.tile([128, 128], F16, tag="pt")
                          nc.tensor.transpose(
                              pt[:], h_sb[:, nt * DH + k * 128: nt * DH + (k + 1) * 128],
                              ident[:])
                          hT = htp.tile([128, 128], F16, tag="hT")
                          nc.vector.tensor_copy(hT[:], pt[:])
                          hTs.append(hT)
                      ps = ps_mm.tile([128, DH], F32, tag="mm")
                      for k in range(cfg.KH):
                          nc.tensor.matmul(ps[:], hTs[k][:], wg_sb[:, l * cfg.KH + k, :],
                                           start=(k == 0), stop=(k == cfg.KH - 1))
                      nc.vector.tensor_scalar(mstg[:, j, :], ps[:],
                                              dinvp_sb[:, nt:nt + 1], None,
                                              mybir.AluOpType.mult)
                  if "no_transform" in opts:
                      nc.vector.memset(mstg[:, :len(nts), :], 0.0)
                  nc.sync.dma_start(
                      cc8_in3[:, sb0:sb0 + len(nts), :],
                      mstg[:, :len(nts), :])
                if "no_ag" not in opts:
                  nc.gpsimd.collective_compute(
                      "AllGather", mybir.AluOpType.bypass,
                      ins=[cc8_in.ap()[:]], outs=[cc8_out.ap()[:]],
                      replica_groups=rg)

              # aggregation
              ci = 0
              for gt in sched["groups"]:
                  # one PSUM bank per dst tile (matmul start= clears the
                  # whole bank, so accumulation groups must not share banks)
                  pbanks = [ps_agg.tile([128, DH], F32, tag="agg",
                                        name=f"agg_g{gt[0]}_{i}")
                            for i in range(len(gt))]

                  for ch in range(cfg.NCHUNK):
                      call = calls[ci + ch]
                      nblk = sum(s[1] for s in call["segs"])
                      gb = gath_pool.tile([128, gmax, DH], F16, tag="gb")
                      if "no_gather" in opts:
                          nc.gpsimd.memset(gb[:, :nblk, :], 0.0)
                      else:
                       nc.gpsimd.dma_gather(
                          gb[:, :nblk, :],
                          cc8_out.ap()[call["chunk"] * cfg.CH_ROWS:
                                       (call["chunk"] + 1) * cfg.CH_ROWS, :],
                          idx_sb[:, call["idx_off"] // 16:
                                 (call["idx_off"] + call["n_idx"]) // 16],
                          call["n_idx"], call["n_idx"], DH,
                          single_packet=False)
                      ss = ss_pool.tile([128, gmax * 128], F16, tag="ss")
                      if "no_onehot" in opts:
                          nc.gpsimd.memset(ss[:, :nblk * 128], 0.0)
                      for q in range(nblk if "no_onehot" not in opts else 0):
                          nc.vector.tensor_scalar(
                              ss[:, q * 128:(q + 1) * 128], iota_sb[:],
                              dl_sb[:, call["s_off"] + q:call["s_off"] + q + 1],
                              None, mybir.AluOpType.is_equal)
                      b = 0
                      for (t, nbt) in call["segs"]:
                          ti = gt.index(t)
                          pb = pbanks[ti][:]
                          for q in range(nbt):
                              if "no_aggmm" in opts:
                                  if ch == 0 and q == 0:
                                      nc.tensor.matmul(pb, ss[:, 0:128],
                                                       gb[:, 0, :],
                                                       start=True, stop=True)
                                  continue
                              nc.tensor.matmul(
                                  pb, ss[:, (b + q) * 128:(b + q + 1) * 128],
                                  gb[:, b + q, :],
                                  start=(ch == 0 and q == 0),
                                  stop=(ch == cfg.NCHUNK - 1 and q == nbt - 1))
                          b += nbt
                  ci += cfg.NCHUNK
                  msl = mself_pool.tile([128, len(gt), DH], F16, tag="msl")
                  nc.sync.dma_start(
                      msl[:], cc8_in3[:, gt[0]:gt[0] + len(gt), :])
                  for ti, t in enumerate(gt):
                      pb = pbanks[ti][:]
                      v2 = vpool.tile([128, DH], F32, tag="v2")
                      nc.vector.tensor_tensor(
                          v2[:], pb, msl[:, ti, :], mybir.AluOpType.add)
                      v = vpool.tile([128, DH], F32, tag="v")
                      nc.vector.scalar_tensor_tensor(
                          v[:], v2[:], dinve_sb[:, t:t + 1], bgb_sb[:, l, :],
                          mybir.AluOpType.mult, mybir.AluOpType.add)
                      nc.scalar.activation(h_sb[:, t * DH:(t + 1) * DH], v[:],
                                           mybir.ActivationFunctionType.Relu)

          # -------- final AllGather of h + pair MLP head --------
          nc.sync.dma_start(
              cc_in.ap().rearrange("(p t) f -> p (t f)", p=128), h_sb[:])
          if "no_ag" not in opts:
              nc.gpsimd.collective_compute(
                  "AllGather", mybir.AluOpType.bypass,
                  ins=[cc_in.ap()[:]], outs=[cc_out.ap()[:]],
                  replica_groups=rg)

          with tc.tile_pool(name="head", bufs=1) as hp:
              pair = hp.tile([128, DH], F16, tag="pair")
              nc.gpsimd.indirect_dma_start(
                  out=pair[:], out_offset=None,
                  in_=cc_out.ap()[:],
                  in_offset=bass.IndirectOffsetOnAxis(ap=pidx_sb[:, 0:1], axis=0))
              # transpose the 32 pair rows: pT[k][:, j] = pair[j, 128k:128k+128]
              pTs = []
              for k in range(2):
                  pt = ps_t.tile([128, 128], F16, tag="pt")
                  nc.tensor.transpose(pt[:, :2 * cfg.B],
                                      pair[0:2 * cfg.B, k * 128:(k + 1) * 128],
                                      ident[0:2 * cfg.B, 0:2 * cfg.B])
                  pT = hp.tile([128, 2 * cfg.B], F16, tag=f"pT{k}")
                  nc.vector.tensor_copy(pT[:], pt[:, :2 * cfg.B])
                  pTs.append(pT)
              # z1 = relu(pair_cat @ W1 + b1): z1T [2][128, B]
              z1T = hp.tile([128, 2, cfg.B], F16, tag="z1T")
              for m in range(2):
                  ps = ps_mm.tile([128, DH], F32, tag="mm")
                  for k in range(4):
                      rhs = pTs[k % 2][:, (k // 2)::2]
                      nc.tensor.matmul(ps[:, :cfg.B], w1_sb[:, k * 2 + m, :], rhs,
                                       start=(k == 0), stop=(k == 3))
                  nc.scalar.activation(z1T[:, m, :], ps[:, :cfg.B],
                                       mybir.ActivationFunctionType.Relu,
                                       bias=b1c_sb[:, m:m + 1])
              z2T = hp.tile([128, cfg.B], F16, tag="z2T")
              ps = ps_mm.tile([128, DH], F32, tag="mm")
              for k in range(2):
                  nc.tensor.matmul(ps[:, :cfg.B], w2_sb[:, k, :], z1T[:, k, :],
                                   start=(k == 0), stop=(k == 1))
              nc.scalar.activation(z2T[:], ps[:, :cfg.B],
                                   mybir.ActivationFunctionType.Relu,
                                   bias=b2c_sb[:, 0:1])
              pz = ps_mm.tile([128, DH], F32, tag="mm")
              nc.tensor.matmul(pz[0:2, :cfg.B], w3_sb[:], z2T[:],
                               start=True, stop=True)
              outv = hp.tile([128, cfg.B], F32, tag="outv")
              nc.vector.tensor_scalar(outv[0:2, :], pz[0:2, :cfg.B],
                                      b3c_sb[0:2, 0:1], None,
                                      mybir.AluOpType.add)
              nc.sync.dma_start(t_out[:], outv[0:2, :])

    nc.compile()
    return nc


class _Runner:
    """Cached PJRT executor: jit once, keep inputs resident on device."""

    def __init__(self, nc, n_cores):
        install_neuronx_cc_hook()
        self.nc = nc
        self.n_cores = n_cores
        pname = nc.partition_id_tensor.name if nc.partition_id_tensor else None
        in_names, out_names, out_avals = [], [], []
        for alloc in nc.m.functions[0].allocations:
            if not isinstance(alloc, mybir.MemoryLocationSet):
                continue
            name = alloc.memorylocations[0].name
            if alloc.kind == "ExternalInput":
                if name != pname:
                    in_names.append(name)
            elif alloc.kind == "ExternalOutput":
                shape = tuple(alloc.tensor_shape)
                dtype = mybir.dt.np(alloc.dtype)
                out_names.append(name)
                out_avals.append(jax.core.ShapedArray(shape, dtype))
        self.in_names = list(in_names)
        self.out_names = out_names
        self.out_avals = out_avals
        n_params = len(in_names)
        all_names = in_names + out_names + ([pname] if pname else [])
        donate = tuple(range(n_params, n_params + len(out_names)))

        def _body(*args):
            operands = list(args)
            if pname is not None:
                operands.append(partition_id_tensor())
            outs = _bass_exec_p.bind(
                *operands, out_avals=tuple(out_avals),
                in_names=tuple(all_names), out_names=tuple(out_names),
                lowering_input_output_aliases=(),
                sim_require_finite=True, sim_require_nnan=True, nc=nc)
            return tuple(outs)

        devices = jax.devices()[:n_cores]
        assert len(devices) == n_cores
        self.mesh = Mesh(np.asarray(devices), ("core",))
        in_specs = (PartitionSpec("core"),) * (n_params + len(out_names))
        out_specs = (PartitionSpec("core"),) * len(out_names)
        self.fn = jax.jit(
            shard_map(_body, mesh=self.mesh, in_specs=in_specs,
                      out_specs=out_specs, check_rep=False),
            donate_argnums=donate, keep_unused=True)
        self.dev_inputs = None

    def set_inputs(self, in_maps):
        sh = NamedSharding(self.mesh, PartitionSpec("core"))
        concat = [np.concatenate([np.asarray(m[name]) for m in in_maps], axis=0)
                  for name in self.in_names]
        self.dev_inputs = [jax.device_put(a, sh) for a in concat]
        for a in self.dev_inputs:
            a.block_until_ready()

    def _zeros(self):
        return [np.zeros((self.n_cores * av.shape[0], *av.shape[1:]), av.dtype)
                for av in self.out_avals]

    def run(self):
        outs = self.fn(*self.dev_inputs, *self._zeros())
        return {name: np.asarray(o) for name, o in zip(self.out_names, outs)}

    def time_exec(self, rounds=3, queue=8):
        """Average per-execution device time over `queue` async dispatches."""
        best = None
        for _ in range(rounds):
            zs = [self._zeros() for _ in range(queue)]
            t0 = time.perf_counter()
            outs = None
            for q in range(queue):
                outs = self.fn(*self.dev_inputs, *zs[q])
            jax.block_until_ready(outs)
            dt = (time.perf_counter() - t0) / queue
            best = dt if best is None else min(best, dt)
        return best


_CACHE = {}


def kernel(**inputs):
    global LAST_EXEC_NS
    cfg = Cfg()
    names = ["x", "edge_index", "batch", "source_ids", "sink_ids",
             "Wi", "bi", "Wg", "bg", "W1", "b1", "W2", "b2", "W3", "b3"]
    arrs = [np.asarray(inputs[n]) for n in names]

    cached = _CACHE.get("entry")
    if cached is not None and all(
            a.shape == b.shape and a.dtype == b.dtype and np.array_equal(a, b)
            for a, b in zip(arrs, cached["arrs"])):
        runner = cached["runner"]
    else:
        in_maps, sched = _preprocess(cfg, inputs)
        key = (cfg.N, cfg.E, sched["NB"], sched["NI"],
               tuple(tuple(r) for r in sched["nb"]))
        if _CACHE.get("build_key") != key:
            _CACHE["nc"] = _build(cfg, sched)
            _CACHE["build_key"] = key
            _CACHE["runner_obj"] = _Runner(_CACHE["nc"], cfg.NC)
        runner = _CACHE["runner_obj"]
        runner.set_inputs(in_maps)
        _CACHE["entry"] = {"arrs": [a.copy() for a in arrs], "runner": runner}
        # warm-up, then per-iteration HW time via repeat-difference: a
        # second program runs REPEAT_R iterations per launch; the delta
        # against the 1-iteration program cancels launch overhead.
        runner.run()
        t1 = runner.time_exec(rounds=5, queue=8)
        REPEAT_R = 5
        if _CACHE.get("build_key_r") != _CACHE["build_key"]:
            _CACHE["runner_r"] = _Runner(
                _build(cfg, sched, repeat=REPEAT_R), cfg.NC)
            _CACHE["build_key_r"] = _CACHE["build_key"]
        runner_r = _CACHE["runner_r"]
        runner_r.set_inputs(in_maps)
        runner_r.run()
        tR = runner_r.time_exec(rounds=5, queue=8)
        per_iter = (tR - t1) / (REPEAT_R - 1)
        if per_iter <= 0:
            per_iter = t1
        _CACHE["entry"]["exec_ns"] = max(1, int(per_iter * 1e9))

    LAST_EXEC_NS = _CACHE["entry"]["exec_ns"]
    res = runner.run()
    return np.ascontiguousarray(res["out"][0:2].T.astype(np.float32))
